# revision 43
# baseline (speedup 1.0000x reference)
"""Fused transformer block (LN + fused QKV/FF proj + MQA attention + SwiGLU FF)
on 8 TRN2 NeuronCores.

Sharding: hybrid DP2 x TP4.
  core c -> batch b = c//4, tensor-parallel shard s = c%4.
  Each core handles its batch's full 2048 tokens in feature-major layout:
    - q: 2 of 8 heads (cols 128*s .. 128*s+128 of the q block)
    - k/v: replicated (width 64 each)
    - ff: 1024 of 4096 cols of both ff_x and gate
    - attn_out / ff_out: matching row shards -> partial [1024, 2048] outputs
  Host sums the 4 partial outputs per batch (row-parallel reduction).

Device layout is feature-major (features on SBUF partitions, tokens on the
free dim) so every matmul contraction is over the partition dim.  gamma and
the q-scale (dim_head**-0.5) are folded into w_fused on the host; per-token
LayerNorm mu/rstd are computed on device via ones-vector matmuls, broadcast
across partitions through a DRAM bounce, mean-subtract applied in place on
x^T, and rstd folded into each projection's PSUM evacuation.
"""

import numpy as np
import ml_dtypes

# ---- problem shapes (hardcoded) ----
B, N, D = 2, 2048, 1024
DH = 64
HEADS = 8
ATTN_INNER = HEADS * DH          # 512
FF_INNER = 4 * D                 # 4096
T = N                            # tokens per core
P = 128
TS = 512
NTS = T // TS                    # 4
NK = D // P                      # 8
NCORES = 8
TP = 4
LH = HEADS // TP                 # 2 local heads
FF_SH = FF_INNER // TP           # 1024
FSH = LH * DH + 2 * DH + 2 * FF_SH   # 2304 packed proj cols per core
NF = FSH // P                    # 18
NKT = T // P                     # 16 key-token tiles

_BF16 = ml_dtypes.bfloat16

_STATE = {}


def _build_nc():
    from concourse import bacc
    import concourse.tile as tile
    from concourse.tile import add_dep_helper
    import concourse.mybir as mybir

    bf16 = mybir.dt.bfloat16
    f32 = mybir.dt.float32
    AF = mybir.ActivationFunctionType

    nc = bacc.Bacc("TRN2", target_bir_lowering=False, debug=False)

    xT_d = nc.dram_tensor("xT", [D, T], bf16, kind="ExternalInput")
    wf_d = nc.dram_tensor("wf", [D, FSH], bf16, kind="ExternalInput")
    wao_d = nc.dram_tensor("wao", [P, D], bf16, kind="ExternalInput")
    wfo_d = nc.dram_tensor("wfo", [D, D], bf16, kind="ExternalInput")
    yT_d = nc.dram_tensor("yT", [D, T], f32, kind="ExternalOutput")
    # DRAM bounce rows for partition-broadcast
    mu_d = nc.dram_tensor("mu_bounce", [1, T], bf16)
    rstd_d = nc.dram_tensor("rstd_bounce", [1, T], f32)

    with tile.TileContext(nc) as tc:
        with (
            tc.tile_pool(name="cp", bufs=1) as cp,
            tc.tile_pool(name="wp", bufs=1) as wp,
            tc.tile_pool(name="acts", bufs=1) as acts,
            tc.tile_pool(name="rows", bufs=1) as rows,
            tc.tile_pool(name="tmp", bufs=3) as tmp,
            tc.tile_pool(name="ps", bufs=1, space="PSUM") as ps,
        ):
            # ---- constants ----
            ones_col = cp.tile([P, 1], bf16)
            nc.vector.memset(ones_col, 1.0)
            ones_row_bf = cp.tile([1, P], bf16)
            nc.vector.memset(ones_row_bf, 1.0)
            f32r = mybir.dt.float32r
            ones_hi = cp.tile([P, 64], f32r)
            nc.vector.memset(ones_hi.bitcast(f32), 1.0)
            ones_row_r = cp.tile([1, P], f32r)
            nc.vector.memset(ones_row_r.bitcast(f32), 1.0)
            eps_t = cp.tile([1, 1], f32)
            nc.vector.memset(eps_t, 1e-5)
            zero_col = cp.tile([P, 1], f32)
            nc.vector.memset(zero_col, 0.0)
            # warm the ACT function tables before the bulk DMAs queue up:
            # lazy table loads otherwise serialize behind ~9MB of weight
            # traffic and stall the first LayerNorm square by ~8us
            warm_o = cp.tile([1, 4], f32)
            for wi, fn in enumerate((AF.Square, AF.Sqrt, AF.Sigmoid, AF.Exp)):
                nc.scalar.activation(warm_o[0:1, wi:wi + 1],
                                     eps_t, fn, bias=zero_col[0:1, :])

            # ---- persistent activations ----
            q2_sb = acts.tile([P, T], bf16)    # q both local heads, feature-major
            kv_sb = acts.tile([P, T], bf16)    # k rows 0-63, v rows 64-127
            k2_sb = acts.tile([P, T], bf16)    # k replica at partitions 64-127
                                               # (head-1 sim row-group packing)
            h_sb = [acts.tile([P, T], bf16, name=f"h{j}") for j in range(NK)]
            out_sb = acts.tile([P, T], bf16)   # attention out, both heads
            mu_b = acts.tile([P, T], bf16)     # mu broadcast
            rstd_b = acts.tile([P, T], f32)    # rstd broadcast
            v_aug = [acts.tile([P, 72], bf16, name=f"va{kt}") for kt in range(NKT)]
            for kt in range(NKT):
                nc.vector.memset(v_aug[kt][:, 64:65], 1.0)

            with tc.tile_pool(name="xp", bufs=1) as xp:
                # ---- load x^T, ts-chunked so stats/proj of slice 0 start
                # as early as possible ----
                xt = [xp.tile([P, T], bf16, name=f"xt{k}") for k in range(NK)]
                for k in range(NK):
                    nc.sync.dma_start(out=xt[k],
                                      in_=xT_d[k * P:(k + 1) * P, :])
                # wf in column-quarters: the first proj chains need only the
                # low column range of every k-tile, so they can start ~10us
                # before the full matrix lands
                wf_sb = [xp.tile([P, FSH], bf16, name=f"wf{k}")
                         for k in range(NK)]
                WQ = FSH // 4
                for q in range(4):
                    qc = slice(q * WQ, (q + 1) * WQ)
                    for k in range(NK):
                        nc.sync.dma_start(out=wf_sb[k][:, qc],
                                          in_=wf_d[k * P:(k + 1) * P, qc])
                # ---- LayerNorm statistics, per token-slice ----
                def emit_stats(ts):
                    col = slice(ts * TS, (ts + 1) * TS)
                    ps_s = ps.tile([1, TS], f32, tag="pp", bufs=5,
                                   name=f"ps_s{ts}")
                    for k in range(NK):
                        nc.tensor.matmul(ps_s, lhsT=ones_col,
                                         rhs=xt[k][:, col],
                                         start=(k == 0), stop=(k == NK - 1))
                    ps_s2 = ps.tile([1, TS], f32, tag="pp", bufs=5,
                                    name=f"ps_s2{ts}")
                    for k in range(NK):
                        x2t = tmp.tile([P, TS], bf16, tag="x2t")
                        nc.vector.tensor_mul(x2t, xt[k][:, col], xt[k][:, col])
                        nc.tensor.matmul(ps_s2, lhsT=ones_col, rhs=x2t,
                                         start=(k == 0), stop=(k == NK - 1))
                    # negvarD = (s^2)/D - s2 = -D*var ;  std = sqrt(-negvarD/D
                    # + eps) ;  mu(bf16) = s/D
                    ssq = rows.tile([1, TS], f32, tag="ssq")
                    nc.scalar.activation(ssq, ps_s, AF.Square,
                         bias=zero_col[0:1, :])
                    mu_bf_r = rows.tile([1, TS], bf16, tag="mu_bf_r")
                    nc.vector.tensor_scalar_mul(mu_bf_r, ps_s, 1.0 / D)
                    negvar = rows.tile([1, TS], f32, tag="negvar")
                    nc.vector.scalar_tensor_tensor(
                        negvar, ssq, 1.0 / D, ps_s2,
                        op0=mybir.AluOpType.mult,
                        op1=mybir.AluOpType.subtract)
                    std = rows.tile([1, TS], f32, tag="std")
                    nc.scalar.activation(std, negvar, AF.Sqrt, bias=eps_t,
                                         scale=-1.0 / D)
                    rstd_r = rows.tile([1, TS],
                                       f32r if ts == 0 else f32,
                                       tag="rstd_r")
                    with nc.allow_low_precision(
                            reason="f32r broadcast operand; ~19-bit "
                                   "mantissa is plenty for rstd"):
                        nc.vector.reciprocal(rstd_r, std)
                    if ts == 0:
                        # slice 0 gates the whole projection phase: broadcast
                        # via K=1 PE matmuls (DMA-free; the DRAM-bounce path
                        # would queue behind the bulk weight loads)
                        pmu = ps.tile([P, TS], f32, tag="pp", bufs=5,
                                      name="pmu0")
                        nc.tensor.matmul(pmu, lhsT=ones_row_bf[0:1, :],
                                         rhs=mu_bf_r, start=True, stop=True)
                        nc.vector.tensor_copy(mu_b[:, col], pmu)
                        prs = ps.tile([P, TS], f32, tag="pp", bufs=5,
                                      name="prs0")
                        nc.tensor.matmul(prs, lhsT=ones_row_r[0:1, :],
                                         rhs=rstd_r,
                                         start=True, stop=True)
                        nc.vector.tensor_copy(rstd_b[:, col], prs)
                    else:
                        # bounce rows through DRAM, broadcast to 128
                        # partitions (gpsimd SWDGE queue; lands during the
                        # previous slice's projection)
                        nc.gpsimd.dma_start(out=mu_d[0:1, col], in_=mu_bf_r)
                        nc.gpsimd.dma_start(out=rstd_d[0:1, col], in_=rstd_r)
                        nc.gpsimd.dma_start(
                            out=mu_b[:, col],
                            in_=mu_d[0:1, col].partition_broadcast(P))
                        nc.gpsimd.dma_start(
                            out=rstd_b[:, col],
                            in_=rstd_d[0:1, col].partition_broadcast(P))

                gate = {}

                def emit_center(ts):
                    # center x in place (emitted separately: this blocks DVE
                    # on the mu broadcast, so it must come after all stats
                    # squares that later PE chains depend on)
                    col = slice(ts * TS, (ts + 1) * TS)
                    for k in range(NK):
                        nc.vector.tensor_sub(xt[k][:, col], xt[k][:, col],
                                             mu_b[:, col])

                # ---- fused projection ----
                # packed col order: [q(128) | kv(128) | (gate_j, ffx_j) x 8]
                def emit_proj(ts):
                    col = slice(ts * TS, (ts + 1) * TS)
                    cur_silu = None
                    for fi in range(NF):
                        pp = ps.tile([P, TS], f32, tag="pp", bufs=5,
                                     name=f"pp{ts}_{fi}")
                        for k in range(NK):
                            nc.tensor.matmul(
                                pp,
                                lhsT=wf_sb[k][:, fi * P:(fi + 1) * P],
                                rhs=xt[k][:, col],
                                start=(k == 0), stop=(k == NK - 1))
                        if fi == 0:
                            nc.vector.tensor_mul(q2_sb[:, col], pp,
                                                 rstd_b[:, col])
                        elif fi == 1:
                            nc.vector.tensor_mul(kv_sb[:, col], pp,
                                                 rstd_b[:, col])
                            # replicate k rows to partitions 64-127 so head-1
                            # sims run in the upper PE row group
                            nc.sync.dma_start(out=k2_sb[64:128, col],
                                               in_=kv_sb[0:64, col])
                        elif fi % 2 == 0:  # gate_j
                            g = tmp.tile([P, TS], bf16, tag="g")
                            nc.vector.tensor_mul(g, pp, rstd_b[:, col])
                            sg = tmp.tile([P, TS], bf16, tag="sg")
                            nc.scalar.activation(sg, g, AF.Sigmoid, bias=zero_col)
                            silu = tmp.tile([P, TS], bf16, tag="silu")
                            nc.vector.tensor_mul(silu, g, sg)
                            cur_silu = silu
                        else:  # ffx_j
                            j = (fi - 3) // 2
                            fx = tmp.tile([P, TS], bf16, tag="fx")
                            nc.vector.tensor_mul(fx, pp, rstd_b[:, col])
                            hmul = nc.vector.tensor_mul(h_sb[j][:, col],
                                                        cur_silu, fx)
                            if ts == 1 and fi == NF - 1:
                                gate["i"] = hmul
                    # v -> token-major, into the v_aug tiles for this slice
                    for kt in range(ts * (TS // P), (ts + 1) * (TS // P)):
                        nc.sync.dma_start(
                            out=v_aug[kt][:, 0:64],
                            in_=kv_sb[64:128, kt * P:(kt + 1) * P],
                            transpose=True)

                # schedule: only stats(0) ahead of proj(0); later slices'
                # stats (and their row math / broadcasts / centering) hide
                # under the previous slice's projection
                emit_stats(0)
                emit_stats(1)
                emit_center(0)
                emit_proj(0)
                emit_stats(2)
                emit_center(1)
                emit_proj(1)
                emit_stats(3)
                emit_center(2)
                emit_proj(2)
                emit_center(3)
                emit_proj(3)
                # output-side weights: needed only ~190us in.  Explicitly
                # gated behind the end of proj slice 1 so the scheduler does
                # not hoist these (dependency-free) DMAs ahead of the x/wf
                # loads and halve the effective prologue load bandwidth.
                wao_sb = wp.tile([P, D], bf16)
                w_in = nc.gpsimd.dma_start(out=wao_sb, in_=wao_d[:, :])
                add_dep_helper(w_in.ins, gate["i"].ins,
                               reason="defer wao load")
                wfo_sb = []
                for k in range(NK):
                    t_ = wp.tile([P, D], bf16, name=f"wfo{k}")
                    w_in = nc.gpsimd.dma_start(out=t_,
                                               in_=wfo_d[k * P:(k + 1) * P, :])
                    add_dep_helper(w_in.ins, gate["i"].ins,
                                   reason="defer wfo load")
                    wfo_sb.append(t_)

            # xp closed: x/wf tiles are dead, reuse SBUF for attention tiles.
            # Attention pipeline over tsq-slots.  Head-0 sims run in PE rows
            # 0-63, head-1 sims concurrently in rows 64-127 (k replicated at
            # partitions 64-127, q head 1 already there).  AV matmuls of the
            # previous slot and y-chain matmuls interleave at ~exp rate so
            # the PE stays busy while ACT churns the exps.
            with (
                tc.tile_pool(name="esp", bufs=44) as esp,
                tc.tile_pool(name="atmp", bufs=2) as atmp,
                tc.tile_pool(name="yp", bufs=3) as yp,
            ):
                es_store = {}
                pavs = {}
                y_chains = []

                def y_chain_gen(tsq, d):
                    qcol = slice(tsq * TS, (tsq + 1) * TS)
                    py = ps.tile([P, TS], f32, tag="pp", bufs=5,
                                 name=f"py{tsq}_{d}")
                    for k in range(NK):
                        nc.tensor.matmul(
                            py, lhsT=wfo_sb[k][:, d * P:(d + 1) * P],
                            rhs=h_sb[k][:, qcol],
                            start=(k == 0), stop=False)
                        yield
                    nc.tensor.matmul(
                        py, lhsT=wao_sb[:, d * P:(d + 1) * P],
                        rhs=out_sb[:, qcol], start=False, stop=True)
                    y_sb = yp.tile([P, TS], f32, tag="ysb",
                                   name=f"ysb{tsq}_{d}")
                    nc.vector.tensor_copy(y_sb, py)
                    nc.gpsimd.dma_start(out=yT_d[d * P:(d + 1) * P, qcol],
                                        in_=y_sb)

                def y_step(n):
                    done = 0
                    while done < n and y_chains:
                        try:
                            next(y_chains[0])
                        except StopIteration:
                            y_chains.pop(0)
                        done += 1

                def emit_sims_pair(tsq, kt):
                    qcol = slice(tsq * TS, (tsq + 1) * TS)
                    kcols = slice(kt * P, (kt + 1) * P)
                    for h in range(LH):
                        psim = ps.tile([P, TS], f32, tag="pp", bufs=5,
                                       name=f"psim{tsq}_{h}_{kt}")
                        if h == 0:
                            nc.tensor.matmul(psim, lhsT=kv_sb[0:64, kcols],
                                             rhs=q2_sb[0:64, qcol],
                                             start=True, stop=True)
                        else:
                            nc.tensor.matmul(psim, lhsT=k2_sb[64:128, kcols],
                                             rhs=q2_sb[64:128, qcol],
                                             start=True, stop=True)
                        es = esp.tile([P, TS], bf16, tag="es",
                                      name=f"es{tsq}_{h}_{kt}")
                        nc.scalar.activation(es, psim, AF.Exp, bias=zero_col)
                        es_store[(tsq, h, kt)] = es

                def av_mm(tsq, h, kt):
                    if kt == 0:
                        pavs[(tsq, h)] = ps.tile([P, TS], f32, tag="pav",
                                                 bufs=3, name=f"pav{tsq}_{h}")
                    nc.tensor.matmul(
                        pavs[(tsq, h)][0:65, :], lhsT=v_aug[kt][:, 0:65],
                        rhs=es_store.pop((tsq, h, kt)),
                        start=(kt == 0), stop=(kt == NKT - 1))

                def emit_av_epilogue(tsq, h):
                    b = tsq * LH + h
                    qcol = slice(tsq * TS, (tsq + 1) * TS)
                    pav = pavs.pop((tsq, h))
                    # denominator (partition 64) -> reciprocal (stays at
                    # partition 64) -> K=1 fp32 PE broadcast over 64 rows,
                    # reading the stationary+moving operands at partition 64
                    rec64 = atmp.tile([P, TS], mybir.dt.float32r,
                                      tag="rec64")
                    with nc.allow_low_precision(
                            reason="f32r broadcast operand; ~19-bit "
                                   "mantissa is plenty for 1/denom"):
                        nc.vector.reciprocal(rec64[64:65, :],
                                             pav[64:65, :])
                    pB = ps.tile([64, TS], f32, tag="pp", bufs=5,
                                 name=f"pB{b}")
                    nc.tensor.matmul(pB, lhsT=ones_hi[64:65, :],
                                     rhs=rec64[64:65, :],
                                     start=True, stop=True)
                    rb = atmp.tile([64, TS], f32, tag="rb")
                    nc.vector.tensor_copy(rb, pB)
                    if h == 0:
                        nc.vector.tensor_mul(out_sb[0:64, qcol],
                                             pav[0:64, :], rb)
                    else:
                        oh1 = atmp.tile([64, TS], bf16, tag="oh1")
                        nc.vector.tensor_mul(oh1, pav[0:64, :], rb)
                        nc.gpsimd.dma_start(out=out_sb[64:128, qcol], in_=oh1)
                        y_chains.extend(y_chain_gen(tsq, d)
                                        for d in range(NK))

                # slot 0: sims only
                for kt in range(NKT):
                    emit_sims_pair(0, kt)
                # slots 1..NTS-1
                for b in range(1, NTS):
                    for kt in range(NKT):
                        emit_sims_pair(b, kt)
                        av_mm(b - 1, 0, kt)
                        if kt == NKT - 1:
                            emit_av_epilogue(b - 1, 0)
                        av_mm(b - 1, 1, kt)
                        y_step(3)
                    emit_av_epilogue(b - 1, 1)
                # final AV slot
                for kt in range(NKT):
                    av_mm(NTS - 1, 0, kt)
                    av_mm(NTS - 1, 1, kt)
                    y_step(3)
                emit_av_epilogue(NTS - 1, 0)
                emit_av_epilogue(NTS - 1, 1)
                y_step(1 << 30)

    nc.compile()
    return nc


def _get_nc():
    if "nc" not in _STATE:
        _STATE["nc"] = _build_nc()
    return _STATE["nc"]


def _prep_inputs(x, gamma, w_fused, w_attn_out, w_ff_out):
    """Host-side shard packing. Returns in_maps for the 8 cores."""
    x = np.asarray(x, dtype=np.float32)
    gamma = np.asarray(gamma, dtype=np.float32)
    w_fused = np.asarray(w_fused, dtype=np.float32)
    w_attn_out = np.asarray(w_attn_out, dtype=np.float32)
    w_ff_out = np.asarray(w_ff_out, dtype=np.float32)

    # fold gamma into w_fused rows; fold q scale into q columns
    wf = w_fused * gamma[:, None]
    wf = wf.copy()
    wf[:, :ATTN_INNER] *= DH ** -0.5

    q_blk = wf[:, :ATTN_INNER]
    k_blk = wf[:, ATTN_INNER:ATTN_INNER + DH]
    v_blk = wf[:, ATTN_INNER + DH:ATTN_INNER + 2 * DH]
    ffx_blk = wf[:, ATTN_INNER + 2 * DH:ATTN_INNER + 2 * DH + FF_INNER]
    gate_blk = wf[:, ATTN_INNER + 2 * DH + FF_INNER:]

    xT = [np.ascontiguousarray(x[b].T).astype(_BF16) for b in range(B)]

    in_maps = []
    for c in range(NCORES):
        b, s = divmod(c, TP)
        cols = [q_blk[:, P * s:P * s + P], k_blk, v_blk]
        for j in range(NK):
            cols.append(gate_blk[:, FF_SH * s + j * P: FF_SH * s + (j + 1) * P])
            cols.append(ffx_blk[:, FF_SH * s + j * P: FF_SH * s + (j + 1) * P])
        wf_c = np.concatenate(cols, axis=1).astype(_BF16)
        wao_c = np.ascontiguousarray(
            w_attn_out[P * s:P * s + P, :]).astype(_BF16)
        wfo_c = np.ascontiguousarray(
            w_ff_out[FF_SH * s:FF_SH * (s + 1), :]).astype(_BF16)
        in_maps.append({"xT": xT[b], "wf": wf_c, "wao": wao_c, "wfo": wfo_c})
    return in_maps


def kernel(x, gamma, w_fused, w_attn_out, w_ff_out):
    import time
    from concourse.bass_utils import run_bass_kernel_spmd

    nc = _get_nc()
    in_maps = _prep_inputs(x, gamma, w_fused, w_attn_out, w_ff_out)

    t0 = time.perf_counter()
    res = run_bass_kernel_spmd(nc, in_maps, core_ids=list(range(NCORES)))
    t1 = time.perf_counter()
    _STATE["last_wall_ns"] = (t1 - t0) * 1e9

    y = np.empty((B, N, D), dtype=np.float32)
    for b in range(B):
        acc = res.results[b * TP]["yT"].astype(np.float32)
        for s in range(1, TP):
            acc = acc + res.results[b * TP + s]["yT"]
        y[b] = acc.T
    return y


# revision 44
# speedup vs baseline: 1.0198x; 1.0198x over previous
"""Fused transformer block (LN + fused QKV/FF proj + MQA attention + SwiGLU FF)
on 8 TRN2 NeuronCores.

Sharding: hybrid DP2 x TP4.
  core c -> batch b = c//4, tensor-parallel shard s = c%4.
  Each core handles its batch's full 2048 tokens in feature-major layout:
    - q: 2 of 8 heads (cols 128*s .. 128*s+128 of the q block)
    - k/v: replicated (width 64 each)
    - ff: 1024 of 4096 cols of both ff_x and gate
    - attn_out / ff_out: matching row shards -> partial [1024, 2048] outputs
  Host sums the 4 partial outputs per batch (row-parallel reduction).

Device layout is feature-major (features on SBUF partitions, tokens on the
free dim) so every matmul contraction is over the partition dim.  gamma and
the q-scale (dim_head**-0.5) are folded into w_fused on the host; per-token
LayerNorm mu/rstd are computed on device via ones-vector matmuls, broadcast
across partitions through a DRAM bounce, mean-subtract applied in place on
x^T, and rstd folded into each projection's PSUM evacuation.
"""

import numpy as np
import ml_dtypes

# ---- problem shapes (hardcoded) ----
B, N, D = 2, 2048, 1024
DH = 64
HEADS = 8
ATTN_INNER = HEADS * DH          # 512
FF_INNER = 4 * D                 # 4096
T = N                            # tokens per core
P = 128
TS = 512
NTS = T // TS                    # 4
NK = D // P                      # 8
NCORES = 8
TP = 4
LH = HEADS // TP                 # 2 local heads
FF_SH = FF_INNER // TP           # 1024
FSH = LH * DH + 2 * DH + 2 * FF_SH   # 2304 packed proj cols per core
NF = FSH // P                    # 18
NKT = T // P                     # 16 key-token tiles

_BF16 = ml_dtypes.bfloat16

_STATE = {}


def _build_nc():
    from concourse import bacc
    import concourse.tile as tile
    from concourse.tile import add_dep_helper
    import concourse.mybir as mybir

    bf16 = mybir.dt.bfloat16
    f32 = mybir.dt.float32
    AF = mybir.ActivationFunctionType

    nc = bacc.Bacc("TRN2", target_bir_lowering=False, debug=False)

    xT_d = nc.dram_tensor("xT", [D, T], bf16, kind="ExternalInput")
    wf_d = nc.dram_tensor("wf", [D, FSH], bf16, kind="ExternalInput")
    wao_d = nc.dram_tensor("wao", [P, D], bf16, kind="ExternalInput")
    wfo_d = nc.dram_tensor("wfo", [D, D], bf16, kind="ExternalInput")
    yT_d = nc.dram_tensor("yT", [D, T], f32, kind="ExternalOutput")
    # DRAM bounce rows for partition-broadcast
    mu_d = nc.dram_tensor("mu_bounce", [1, T], bf16)
    rstd_d = nc.dram_tensor("rstd_bounce", [1, T], f32)

    with tile.TileContext(nc) as tc:
        with (
            tc.tile_pool(name="cp", bufs=1) as cp,
            tc.tile_pool(name="wp", bufs=1) as wp,
            tc.tile_pool(name="acts", bufs=1) as acts,
            tc.tile_pool(name="rows", bufs=1) as rows,
            tc.tile_pool(name="tmp", bufs=3) as tmp,
            tc.tile_pool(name="ps", bufs=1, space="PSUM") as ps,
        ):
            # ---- constants ----
            ones_col = cp.tile([P, 1], bf16)
            nc.vector.memset(ones_col, 1.0)
            ones_row_bf = cp.tile([1, P], bf16)
            nc.vector.memset(ones_row_bf, 1.0)
            f32r = mybir.dt.float32r
            ones_hi = cp.tile([P, 64], f32r)
            nc.vector.memset(ones_hi.bitcast(f32), 1.0)
            ones_row_r = cp.tile([1, P], f32r)
            nc.vector.memset(ones_row_r.bitcast(f32), 1.0)
            eps_t = cp.tile([1, 1], f32)
            nc.vector.memset(eps_t, 1e-5)
            zero_col = cp.tile([P, 1], f32)
            nc.vector.memset(zero_col, 0.0)
            # warm the ACT function tables before the bulk DMAs queue up:
            # lazy table loads otherwise serialize behind ~9MB of weight
            # traffic and stall the first LayerNorm square by ~8us
            warm_o = cp.tile([1, 4], f32)
            for wi, fn in enumerate((AF.Square, AF.Sqrt, AF.Sigmoid, AF.Exp)):
                nc.scalar.activation(warm_o[0:1, wi:wi + 1],
                                     eps_t, fn, bias=zero_col[0:1, :])

            # ---- persistent activations ----
            q2_sb = acts.tile([P, T], bf16)    # q both local heads, feature-major
            kv_sb = acts.tile([P, T], bf16)    # k rows 0-63, v rows 64-127
            k2_sb = acts.tile([P, T], bf16)    # k replica at partitions 64-127
                                               # (head-1 sim row-group packing)
            h_sb = [acts.tile([P, T], bf16, name=f"h{j}") for j in range(NK)]
            out_sb = acts.tile([P, T], bf16)   # attention out, both heads
            mu_b = acts.tile([P, T], bf16)     # mu broadcast
            rstd_b = acts.tile([P, T], f32)    # rstd broadcast
            v_aug = [acts.tile([P, 72], bf16, name=f"va{kt}") for kt in range(NKT)]
            for kt in range(NKT):
                nc.vector.memset(v_aug[kt][:, 64:65], 1.0)

            with tc.tile_pool(name="xp", bufs=1) as xp:
                # ---- load x^T, ts-chunked so stats/proj of slice 0 start
                # as early as possible ----
                # x in two column-halves and wf in column-quarters,
                # interleaved so that (a) slice-0/1 stats are gated on only
                # half of x and (b) the first proj chains are gated on only
                # the first quarter of wf -> the projection starts ~10us
                # earlier than with monolithic loads
                xt = [xp.tile([P, T], bf16, name=f"xt{k}") for k in range(NK)]
                wf_sb = [xp.tile([P, FSH], bf16, name=f"wf{k}")
                         for k in range(NK)]
                WQ = FSH // 4
                xh1 = slice(0, T // 2)
                xh2 = slice(T // 2, T)
                for k in range(NK):
                    nc.sync.dma_start(out=xt[k][:, xh1],
                                      in_=xT_d[k * P:(k + 1) * P, xh1])
                for q in range(2):
                    qc = slice(q * WQ, (q + 1) * WQ)
                    for k in range(NK):
                        nc.sync.dma_start(out=wf_sb[k][:, qc],
                                          in_=wf_d[k * P:(k + 1) * P, qc])
                for k in range(NK):
                    nc.sync.dma_start(out=xt[k][:, xh2],
                                      in_=xT_d[k * P:(k + 1) * P, xh2])
                for q in range(2, 4):
                    qc = slice(q * WQ, (q + 1) * WQ)
                    for k in range(NK):
                        nc.sync.dma_start(out=wf_sb[k][:, qc],
                                          in_=wf_d[k * P:(k + 1) * P, qc])
                # ---- LayerNorm statistics, per token-slice ----
                def emit_stats(ts):
                    col = slice(ts * TS, (ts + 1) * TS)
                    ps_s = ps.tile([1, TS], f32, tag="pp", bufs=5,
                                   name=f"ps_s{ts}")
                    for k in range(NK):
                        nc.tensor.matmul(ps_s, lhsT=ones_col,
                                         rhs=xt[k][:, col],
                                         start=(k == 0), stop=(k == NK - 1))
                    ps_s2 = ps.tile([1, TS], f32, tag="pp", bufs=5,
                                    name=f"ps_s2{ts}")
                    for k in range(NK):
                        x2t = tmp.tile([P, TS], bf16, tag="x2t")
                        nc.vector.tensor_mul(x2t, xt[k][:, col], xt[k][:, col])
                        nc.tensor.matmul(ps_s2, lhsT=ones_col, rhs=x2t,
                                         start=(k == 0), stop=(k == NK - 1))
                    # negvarD = (s^2)/D - s2 = -D*var ;  std = sqrt(-negvarD/D
                    # + eps) ;  mu(bf16) = s/D
                    ssq = rows.tile([1, TS], f32, tag="ssq")
                    nc.scalar.activation(ssq, ps_s, AF.Square,
                         bias=zero_col[0:1, :])
                    mu_bf_r = rows.tile([1, TS], bf16, tag="mu_bf_r")
                    nc.vector.tensor_scalar_mul(mu_bf_r, ps_s, 1.0 / D)
                    negvar = rows.tile([1, TS], f32, tag="negvar")
                    nc.vector.scalar_tensor_tensor(
                        negvar, ssq, 1.0 / D, ps_s2,
                        op0=mybir.AluOpType.mult,
                        op1=mybir.AluOpType.subtract)
                    std = rows.tile([1, TS], f32, tag="std")
                    nc.scalar.activation(std, negvar, AF.Sqrt, bias=eps_t,
                                         scale=-1.0 / D)
                    rstd_r = rows.tile([1, TS],
                                       f32r if ts == 0 else f32,
                                       tag="rstd_r")
                    with nc.allow_low_precision(
                            reason="f32r broadcast operand; ~19-bit "
                                   "mantissa is plenty for rstd"):
                        nc.vector.reciprocal(rstd_r, std)
                    if ts == 0:
                        # slice 0 gates the whole projection phase: broadcast
                        # via K=1 PE matmuls (DMA-free; the DRAM-bounce path
                        # would queue behind the bulk weight loads)
                        pmu = ps.tile([P, TS], f32, tag="pp", bufs=5,
                                      name="pmu0")
                        nc.tensor.matmul(pmu, lhsT=ones_row_bf[0:1, :],
                                         rhs=mu_bf_r, start=True, stop=True)
                        nc.vector.tensor_copy(mu_b[:, col], pmu)
                        prs = ps.tile([P, TS], f32, tag="pp", bufs=5,
                                      name="prs0")
                        nc.tensor.matmul(prs, lhsT=ones_row_r[0:1, :],
                                         rhs=rstd_r,
                                         start=True, stop=True)
                        nc.vector.tensor_copy(rstd_b[:, col], prs)
                    else:
                        # bounce rows through DRAM, broadcast to 128
                        # partitions (gpsimd SWDGE queue; lands during the
                        # previous slice's projection)
                        nc.gpsimd.dma_start(out=mu_d[0:1, col], in_=mu_bf_r)
                        nc.gpsimd.dma_start(out=rstd_d[0:1, col], in_=rstd_r)
                        nc.gpsimd.dma_start(
                            out=mu_b[:, col],
                            in_=mu_d[0:1, col].partition_broadcast(P))
                        nc.gpsimd.dma_start(
                            out=rstd_b[:, col],
                            in_=rstd_d[0:1, col].partition_broadcast(P))

                gate = {}

                def emit_center(ts):
                    # center x in place (emitted separately: this blocks DVE
                    # on the mu broadcast, so it must come after all stats
                    # squares that later PE chains depend on)
                    col = slice(ts * TS, (ts + 1) * TS)
                    for k in range(NK):
                        nc.vector.tensor_sub(xt[k][:, col], xt[k][:, col],
                                             mu_b[:, col])

                # ---- fused projection ----
                # packed col order: [q(128) | kv(128) | (gate_j, ffx_j) x 8]
                def emit_proj(ts):
                    col = slice(ts * TS, (ts + 1) * TS)
                    cur_silu = None
                    for fi in range(NF):
                        pp = ps.tile([P, TS], f32, tag="pp", bufs=5,
                                     name=f"pp{ts}_{fi}")
                        for k in range(NK):
                            nc.tensor.matmul(
                                pp,
                                lhsT=wf_sb[k][:, fi * P:(fi + 1) * P],
                                rhs=xt[k][:, col],
                                start=(k == 0), stop=(k == NK - 1))
                        if fi == 0:
                            nc.vector.tensor_mul(q2_sb[:, col], pp,
                                                 rstd_b[:, col])
                        elif fi == 1:
                            nc.vector.tensor_mul(kv_sb[:, col], pp,
                                                 rstd_b[:, col])
                            # replicate k rows to partitions 64-127 so head-1
                            # sims run in the upper PE row group
                            nc.sync.dma_start(out=k2_sb[64:128, col],
                                               in_=kv_sb[0:64, col])
                        elif fi % 2 == 0:  # gate_j
                            g = tmp.tile([P, TS], bf16, tag="g")
                            nc.vector.tensor_mul(g, pp, rstd_b[:, col])
                            sg = tmp.tile([P, TS], bf16, tag="sg")
                            nc.scalar.activation(sg, g, AF.Sigmoid, bias=zero_col)
                            silu = tmp.tile([P, TS], bf16, tag="silu")
                            nc.vector.tensor_mul(silu, g, sg)
                            cur_silu = silu
                        else:  # ffx_j
                            j = (fi - 3) // 2
                            fx = tmp.tile([P, TS], bf16, tag="fx")
                            nc.vector.tensor_mul(fx, pp, rstd_b[:, col])
                            hmul = nc.vector.tensor_mul(h_sb[j][:, col],
                                                        cur_silu, fx)
                            if ts == 1 and fi == NF - 1:
                                gate["i"] = hmul
                    # v -> token-major, into the v_aug tiles for this slice
                    for kt in range(ts * (TS // P), (ts + 1) * (TS // P)):
                        nc.sync.dma_start(
                            out=v_aug[kt][:, 0:64],
                            in_=kv_sb[64:128, kt * P:(kt + 1) * P],
                            transpose=True)

                # schedule: only stats(0) ahead of proj(0); later slices'
                # stats (and their row math / broadcasts / centering) hide
                # under the previous slice's projection
                emit_stats(0)
                emit_stats(1)
                emit_center(0)
                emit_proj(0)
                emit_stats(2)
                emit_center(1)
                emit_proj(1)
                emit_stats(3)
                emit_center(2)
                emit_proj(2)
                emit_center(3)
                emit_proj(3)
                # output-side weights: needed only ~190us in.  Explicitly
                # gated behind the end of proj slice 1 so the scheduler does
                # not hoist these (dependency-free) DMAs ahead of the x/wf
                # loads and halve the effective prologue load bandwidth.
                wao_sb = wp.tile([P, D], bf16)
                w_in = nc.gpsimd.dma_start(out=wao_sb, in_=wao_d[:, :])
                add_dep_helper(w_in.ins, gate["i"].ins,
                               reason="defer wao load")
                wfo_sb = []
                for k in range(NK):
                    t_ = wp.tile([P, D], bf16, name=f"wfo{k}")
                    w_in = nc.gpsimd.dma_start(out=t_,
                                               in_=wfo_d[k * P:(k + 1) * P, :])
                    add_dep_helper(w_in.ins, gate["i"].ins,
                                   reason="defer wfo load")
                    wfo_sb.append(t_)

            # xp closed: x/wf tiles are dead, reuse SBUF for attention tiles.
            # Attention pipeline over tsq-slots.  Head-0 sims run in PE rows
            # 0-63, head-1 sims concurrently in rows 64-127 (k replicated at
            # partitions 64-127, q head 1 already there).  AV matmuls of the
            # previous slot and y-chain matmuls interleave at ~exp rate so
            # the PE stays busy while ACT churns the exps.
            with (
                tc.tile_pool(name="esp", bufs=44) as esp,
                tc.tile_pool(name="atmp", bufs=2) as atmp,
                tc.tile_pool(name="yp", bufs=3) as yp,
            ):
                es_store = {}
                pavs = {}
                y_chains = []

                def y_chain_gen(tsq, d):
                    qcol = slice(tsq * TS, (tsq + 1) * TS)
                    py = ps.tile([P, TS], f32, tag="pp", bufs=5,
                                 name=f"py{tsq}_{d}")
                    for k in range(NK):
                        nc.tensor.matmul(
                            py, lhsT=wfo_sb[k][:, d * P:(d + 1) * P],
                            rhs=h_sb[k][:, qcol],
                            start=(k == 0), stop=False)
                        yield
                    nc.tensor.matmul(
                        py, lhsT=wao_sb[:, d * P:(d + 1) * P],
                        rhs=out_sb[:, qcol], start=False, stop=True)
                    y_sb = yp.tile([P, TS], f32, tag="ysb",
                                   name=f"ysb{tsq}_{d}")
                    nc.vector.tensor_copy(y_sb, py)
                    nc.gpsimd.dma_start(out=yT_d[d * P:(d + 1) * P, qcol],
                                        in_=y_sb)

                def y_step(n):
                    done = 0
                    while done < n and y_chains:
                        try:
                            next(y_chains[0])
                        except StopIteration:
                            y_chains.pop(0)
                        done += 1

                def emit_sims_pair(tsq, kt):
                    qcol = slice(tsq * TS, (tsq + 1) * TS)
                    kcols = slice(kt * P, (kt + 1) * P)
                    for h in range(LH):
                        psim = ps.tile([P, TS], f32, tag="pp", bufs=5,
                                       name=f"psim{tsq}_{h}_{kt}")
                        if h == 0:
                            nc.tensor.matmul(psim, lhsT=kv_sb[0:64, kcols],
                                             rhs=q2_sb[0:64, qcol],
                                             start=True, stop=True)
                        else:
                            nc.tensor.matmul(psim, lhsT=k2_sb[64:128, kcols],
                                             rhs=q2_sb[64:128, qcol],
                                             start=True, stop=True)
                        es = esp.tile([P, TS], bf16, tag="es",
                                      name=f"es{tsq}_{h}_{kt}")
                        nc.scalar.activation(es, psim, AF.Exp, bias=zero_col)
                        es_store[(tsq, h, kt)] = es

                def av_mm(tsq, h, kt):
                    if kt == 0:
                        pavs[(tsq, h)] = ps.tile([P, TS], f32, tag="pav",
                                                 bufs=3, name=f"pav{tsq}_{h}")
                    nc.tensor.matmul(
                        pavs[(tsq, h)][0:65, :], lhsT=v_aug[kt][:, 0:65],
                        rhs=es_store.pop((tsq, h, kt)),
                        start=(kt == 0), stop=(kt == NKT - 1))

                def emit_av_epilogue(tsq, h):
                    b = tsq * LH + h
                    qcol = slice(tsq * TS, (tsq + 1) * TS)
                    pav = pavs.pop((tsq, h))
                    # denominator (partition 64) -> reciprocal (stays at
                    # partition 64) -> K=1 fp32 PE broadcast over 64 rows,
                    # reading the stationary+moving operands at partition 64
                    rec64 = atmp.tile([P, TS], mybir.dt.float32r,
                                      tag="rec64")
                    with nc.allow_low_precision(
                            reason="f32r broadcast operand; ~19-bit "
                                   "mantissa is plenty for 1/denom"):
                        nc.vector.reciprocal(rec64[64:65, :],
                                             pav[64:65, :])
                    pB = ps.tile([64, TS], f32, tag="pp", bufs=5,
                                 name=f"pB{b}")
                    nc.tensor.matmul(pB, lhsT=ones_hi[64:65, :],
                                     rhs=rec64[64:65, :],
                                     start=True, stop=True)
                    rb = atmp.tile([64, TS], f32, tag="rb")
                    nc.vector.tensor_copy(rb, pB)
                    if h == 0:
                        nc.vector.tensor_mul(out_sb[0:64, qcol],
                                             pav[0:64, :], rb)
                    else:
                        oh1 = atmp.tile([64, TS], bf16, tag="oh1")
                        nc.vector.tensor_mul(oh1, pav[0:64, :], rb)
                        nc.gpsimd.dma_start(out=out_sb[64:128, qcol], in_=oh1)
                        y_chains.extend(y_chain_gen(tsq, d)
                                        for d in range(NK))

                # slot 0: sims only
                for kt in range(NKT):
                    emit_sims_pair(0, kt)
                # slots 1..NTS-1
                for b in range(1, NTS):
                    for kt in range(NKT):
                        emit_sims_pair(b, kt)
                        av_mm(b - 1, 0, kt)
                        if kt == NKT - 1:
                            emit_av_epilogue(b - 1, 0)
                        av_mm(b - 1, 1, kt)
                        y_step(3)
                    emit_av_epilogue(b - 1, 1)
                # final AV slot
                for kt in range(NKT):
                    av_mm(NTS - 1, 0, kt)
                    av_mm(NTS - 1, 1, kt)
                    y_step(3)
                emit_av_epilogue(NTS - 1, 0)
                emit_av_epilogue(NTS - 1, 1)
                y_step(1 << 30)

    nc.compile()
    return nc


def _get_nc():
    if "nc" not in _STATE:
        _STATE["nc"] = _build_nc()
    return _STATE["nc"]


def _prep_inputs(x, gamma, w_fused, w_attn_out, w_ff_out):
    """Host-side shard packing. Returns in_maps for the 8 cores."""
    x = np.asarray(x, dtype=np.float32)
    gamma = np.asarray(gamma, dtype=np.float32)
    w_fused = np.asarray(w_fused, dtype=np.float32)
    w_attn_out = np.asarray(w_attn_out, dtype=np.float32)
    w_ff_out = np.asarray(w_ff_out, dtype=np.float32)

    # fold gamma into w_fused rows; fold q scale into q columns
    wf = w_fused * gamma[:, None]
    wf = wf.copy()
    wf[:, :ATTN_INNER] *= DH ** -0.5

    q_blk = wf[:, :ATTN_INNER]
    k_blk = wf[:, ATTN_INNER:ATTN_INNER + DH]
    v_blk = wf[:, ATTN_INNER + DH:ATTN_INNER + 2 * DH]
    ffx_blk = wf[:, ATTN_INNER + 2 * DH:ATTN_INNER + 2 * DH + FF_INNER]
    gate_blk = wf[:, ATTN_INNER + 2 * DH + FF_INNER:]

    xT = [np.ascontiguousarray(x[b].T).astype(_BF16) for b in range(B)]

    in_maps = []
    for c in range(NCORES):
        b, s = divmod(c, TP)
        cols = [q_blk[:, P * s:P * s + P], k_blk, v_blk]
        for j in range(NK):
            cols.append(gate_blk[:, FF_SH * s + j * P: FF_SH * s + (j + 1) * P])
            cols.append(ffx_blk[:, FF_SH * s + j * P: FF_SH * s + (j + 1) * P])
        wf_c = np.concatenate(cols, axis=1).astype(_BF16)
        wao_c = np.ascontiguousarray(
            w_attn_out[P * s:P * s + P, :]).astype(_BF16)
        wfo_c = np.ascontiguousarray(
            w_ff_out[FF_SH * s:FF_SH * (s + 1), :]).astype(_BF16)
        in_maps.append({"xT": xT[b], "wf": wf_c, "wao": wao_c, "wfo": wfo_c})
    return in_maps


def kernel(x, gamma, w_fused, w_attn_out, w_ff_out):
    import time
    from concourse.bass_utils import run_bass_kernel_spmd

    nc = _get_nc()
    in_maps = _prep_inputs(x, gamma, w_fused, w_attn_out, w_ff_out)

    t0 = time.perf_counter()
    res = run_bass_kernel_spmd(nc, in_maps, core_ids=list(range(NCORES)))
    t1 = time.perf_counter()
    _STATE["last_wall_ns"] = (t1 - t0) * 1e9

    y = np.empty((B, N, D), dtype=np.float32)
    for b in range(B):
        acc = res.results[b * TP]["yT"].astype(np.float32)
        for s in range(1, TP):
            acc = acc + res.results[b * TP + s]["yT"]
        y[b] = acc.T
    return y


# revision 45
# speedup vs baseline: 1.0412x; 1.0210x over previous
"""Fused transformer block (LN + fused QKV/FF proj + MQA attention + SwiGLU FF)
on 8 TRN2 NeuronCores.

Sharding: hybrid DP2 x TP4.
  core c -> batch b = c//4, tensor-parallel shard s = c%4.
  Each core handles its batch's full 2048 tokens in feature-major layout:
    - q: 2 of 8 heads (cols 128*s .. 128*s+128 of the q block)
    - k/v: replicated (width 64 each)
    - ff: 1024 of 4096 cols of both ff_x and gate
    - attn_out / ff_out: matching row shards -> partial [1024, 2048] outputs
  Host sums the 4 partial outputs per batch (row-parallel reduction).

Device layout is feature-major (features on SBUF partitions, tokens on the
free dim) so every matmul contraction is over the partition dim.  gamma and
the q-scale (dim_head**-0.5) are folded into w_fused on the host; per-token
LayerNorm mu/rstd are computed on device via ones-vector matmuls, broadcast
across partitions through a DRAM bounce, mean-subtract applied in place on
x^T, and rstd folded into each projection's PSUM evacuation.
"""

import numpy as np
import ml_dtypes

# ---- problem shapes (hardcoded) ----
B, N, D = 2, 2048, 1024
DH = 64
HEADS = 8
ATTN_INNER = HEADS * DH          # 512
FF_INNER = 4 * D                 # 4096
T = N                            # tokens per core
P = 128
TS = 512
NTS = T // TS                    # 4
NK = D // P                      # 8
NCORES = 8
TP = 4
LH = HEADS // TP                 # 2 local heads
FF_SH = FF_INNER // TP           # 1024
FSH = LH * DH + 2 * DH + 2 * FF_SH   # 2304 packed proj cols per core
NF = FSH // P                    # 18
NKT = T // P                     # 16 key-token tiles

_BF16 = ml_dtypes.bfloat16

_STATE = {}


def _build_nc():
    from concourse import bacc
    import concourse.tile as tile
    from concourse.tile import add_dep_helper
    import concourse.mybir as mybir

    bf16 = mybir.dt.bfloat16
    f32 = mybir.dt.float32
    AF = mybir.ActivationFunctionType

    nc = bacc.Bacc("TRN2", target_bir_lowering=False, debug=False)

    xT_d = nc.dram_tensor("xT", [D, T], bf16, kind="ExternalInput")
    wf_d = nc.dram_tensor("wf", [D, FSH], bf16, kind="ExternalInput")
    wao_d = nc.dram_tensor("wao", [P, D], bf16, kind="ExternalInput")
    wfo_d = nc.dram_tensor("wfo", [D, D], bf16, kind="ExternalInput")
    yT_d = nc.dram_tensor("yT", [D, T], f32, kind="ExternalOutput")
    # DRAM bounce rows for partition-broadcast
    mu_d = nc.dram_tensor("mu_bounce", [1, T], bf16)
    rstd_d = nc.dram_tensor("rstd_bounce", [1, T], f32)

    with tile.TileContext(nc) as tc:
        with (
            tc.tile_pool(name="cp", bufs=1) as cp,
            tc.tile_pool(name="wp", bufs=1) as wp,
            tc.tile_pool(name="acts", bufs=1) as acts,
            tc.tile_pool(name="rows", bufs=1) as rows,
            tc.tile_pool(name="tmp", bufs=3) as tmp,
            tc.tile_pool(name="ps", bufs=1, space="PSUM") as ps,
        ):
            # ---- constants ----
            ones_col = cp.tile([P, 1], bf16)
            nc.vector.memset(ones_col, 1.0)
            ones_row_bf = cp.tile([1, P], bf16)
            nc.vector.memset(ones_row_bf, 1.0)
            f32r = mybir.dt.float32r
            ones_hi = cp.tile([P, 64], f32r)
            nc.vector.memset(ones_hi.bitcast(f32), 1.0)
            ones_row_r = cp.tile([1, P], f32r)
            nc.vector.memset(ones_row_r.bitcast(f32), 1.0)
            eps_t = cp.tile([1, 1], f32)
            nc.vector.memset(eps_t, 1e-5)
            zero_col = cp.tile([P, 1], f32)
            nc.vector.memset(zero_col, 0.0)
            # warm the ACT function tables before the bulk DMAs queue up:
            # lazy table loads otherwise serialize behind ~9MB of weight
            # traffic and stall the first LayerNorm square by ~8us
            warm_o = cp.tile([1, 4], f32)
            for wi, fn in enumerate((AF.Square, AF.Sqrt, AF.Sigmoid, AF.Exp)):
                nc.scalar.activation(warm_o[0:1, wi:wi + 1],
                                     eps_t, fn, bias=zero_col[0:1, :])

            # ---- persistent activations ----
            q2_sb = acts.tile([P, T], bf16)    # q both local heads, feature-major
            kv_sb = acts.tile([P, T], bf16)    # k rows 0-63, v rows 64-127
            k2_sb = acts.tile([P, T], bf16)    # k replica at partitions 64-127
                                               # (head-1 sim row-group packing)
            h_sb = [acts.tile([P, T], bf16, name=f"h{j}") for j in range(NK)]
            out_sb = acts.tile([P, T], bf16)   # attention out, both heads
            mu_b = acts.tile([P, T], bf16)     # mu broadcast
            rstd_b = acts.tile([P, T], f32)    # rstd broadcast
            v_aug = [acts.tile([P, 72], bf16, name=f"va{kt}") for kt in range(NKT)]
            for kt in range(NKT):
                nc.vector.memset(v_aug[kt][:, 64:65], 1.0)

            with tc.tile_pool(name="xp", bufs=1) as xp:
                # ---- load x^T, ts-chunked so stats/proj of slice 0 start
                # as early as possible ----
                # x in two column-halves and wf in column-quarters,
                # interleaved so that (a) slice-0/1 stats are gated on only
                # half of x and (b) the first proj chains are gated on only
                # the first quarter of wf -> the projection starts ~10us
                # earlier than with monolithic loads
                xt = [xp.tile([P, T], bf16, name=f"xt{k}") for k in range(NK)]
                wf_sb = [xp.tile([P, FSH], bf16, name=f"wf{k}")
                         for k in range(NK)]
                WQ = FSH // 4
                xh1 = slice(0, T // 2)
                xh2 = slice(T // 2, T)
                for k in range(NK):
                    nc.sync.dma_start(out=xt[k][:, xh1],
                                      in_=xT_d[k * P:(k + 1) * P, xh1])
                for q in range(2):
                    qc = slice(q * WQ, (q + 1) * WQ)
                    for k in range(NK):
                        nc.sync.dma_start(out=wf_sb[k][:, qc],
                                          in_=wf_d[k * P:(k + 1) * P, qc])
                for k in range(NK):
                    nc.sync.dma_start(out=xt[k][:, xh2],
                                      in_=xT_d[k * P:(k + 1) * P, xh2])
                for q in range(2, 4):
                    qc = slice(q * WQ, (q + 1) * WQ)
                    for k in range(NK):
                        nc.sync.dma_start(out=wf_sb[k][:, qc],
                                          in_=wf_d[k * P:(k + 1) * P, qc])
                # ---- LayerNorm statistics, per token-slice ----
                def emit_stats(ts):
                    col = slice(ts * TS, (ts + 1) * TS)
                    ps_s = ps.tile([1, TS], f32, tag="pp", bufs=5,
                                   name=f"ps_s{ts}")
                    for k in range(NK):
                        nc.tensor.matmul(ps_s, lhsT=ones_col,
                                         rhs=xt[k][:, col],
                                         start=(k == 0), stop=(k == NK - 1))
                    ps_s2 = ps.tile([1, TS], f32, tag="pp", bufs=5,
                                    name=f"ps_s2{ts}")
                    for k in range(NK):
                        x2t = tmp.tile([P, TS], bf16, tag="x2t")
                        nc.vector.tensor_mul(x2t, xt[k][:, col], xt[k][:, col])
                        nc.tensor.matmul(ps_s2, lhsT=ones_col, rhs=x2t,
                                         start=(k == 0), stop=(k == NK - 1))
                    # negvarD = (s^2)/D - s2 = -D*var ;  std = sqrt(-negvarD/D
                    # + eps) ;  mu(bf16) = s/D
                    ssq = rows.tile([1, TS], f32, tag="ssq")
                    nc.scalar.activation(ssq, ps_s, AF.Square,
                         bias=zero_col[0:1, :])
                    mu_bf_r = rows.tile([1, TS], bf16, tag="mu_bf_r")
                    nc.vector.tensor_scalar_mul(mu_bf_r, ps_s, 1.0 / D)
                    negvar = rows.tile([1, TS], f32, tag="negvar")
                    nc.vector.scalar_tensor_tensor(
                        negvar, ssq, 1.0 / D, ps_s2,
                        op0=mybir.AluOpType.mult,
                        op1=mybir.AluOpType.subtract)
                    std = rows.tile([1, TS], f32, tag="std")
                    nc.scalar.activation(std, negvar, AF.Sqrt, bias=eps_t,
                                         scale=-1.0 / D)
                    rstd_r = rows.tile([1, TS],
                                       f32r if ts == 0 else f32,
                                       tag="rstd_r")
                    with nc.allow_low_precision(
                            reason="f32r broadcast operand; ~19-bit "
                                   "mantissa is plenty for rstd"):
                        nc.vector.reciprocal(rstd_r, std)
                    if ts == 0:
                        # slice 0 gates the whole projection phase: broadcast
                        # via K=1 PE matmuls (DMA-free; the DRAM-bounce path
                        # would queue behind the bulk weight loads)
                        pmu = ps.tile([P, TS], f32, tag="pp", bufs=5,
                                      name="pmu0")
                        nc.tensor.matmul(pmu, lhsT=ones_row_bf[0:1, :],
                                         rhs=mu_bf_r, start=True, stop=True)
                        nc.vector.tensor_copy(mu_b[:, col], pmu)
                        prs = ps.tile([P, TS], f32, tag="pp", bufs=5,
                                      name="prs0")
                        nc.tensor.matmul(prs, lhsT=ones_row_r[0:1, :],
                                         rhs=rstd_r,
                                         start=True, stop=True)
                        nc.vector.tensor_copy(rstd_b[:, col], prs)
                    else:
                        # bounce rows through DRAM, broadcast to 128
                        # partitions (gpsimd SWDGE queue; lands during the
                        # previous slice's projection)
                        nc.gpsimd.dma_start(out=mu_d[0:1, col], in_=mu_bf_r)
                        nc.gpsimd.dma_start(out=rstd_d[0:1, col], in_=rstd_r)
                        nc.gpsimd.dma_start(
                            out=mu_b[:, col],
                            in_=mu_d[0:1, col].partition_broadcast(P))
                        nc.gpsimd.dma_start(
                            out=rstd_b[:, col],
                            in_=rstd_d[0:1, col].partition_broadcast(P))

                gate = {}

                def emit_center(ts):
                    # center x in place (emitted separately: this blocks DVE
                    # on the mu broadcast, so it must come after all stats
                    # squares that later PE chains depend on)
                    col = slice(ts * TS, (ts + 1) * TS)
                    for k in range(NK):
                        nc.vector.tensor_sub(xt[k][:, col], xt[k][:, col],
                                             mu_b[:, col])

                # ---- fused projection ----
                # packed col order: [q(128) | kv(128) | (gate_j, ffx_j) x 8]
                def emit_proj(ts):
                    col = slice(ts * TS, (ts + 1) * TS)
                    cur_silu = None
                    for fi in range(NF):
                        pp = ps.tile([P, TS], f32, tag="pp", bufs=5,
                                     name=f"pp{ts}_{fi}")
                        for k in range(NK):
                            nc.tensor.matmul(
                                pp,
                                lhsT=wf_sb[k][:, fi * P:(fi + 1) * P],
                                rhs=xt[k][:, col],
                                start=(k == 0), stop=(k == NK - 1))
                        if fi == 0:
                            nc.vector.tensor_mul(q2_sb[:, col], pp,
                                                 rstd_b[:, col])
                        elif fi == 1:
                            nc.vector.tensor_mul(kv_sb[:, col], pp,
                                                 rstd_b[:, col])
                            # replicate k rows to partitions 64-127 so head-1
                            # sims run in the upper PE row group
                            nc.sync.dma_start(out=k2_sb[64:128, col],
                                               in_=kv_sb[0:64, col])
                        elif fi % 2 == 0:  # gate_j
                            g = tmp.tile([P, TS], bf16, tag="g")
                            nc.vector.tensor_mul(g, pp, rstd_b[:, col])
                            sg = tmp.tile([P, TS], bf16, tag="sg")
                            nc.scalar.activation(sg, g, AF.Sigmoid, bias=zero_col)
                            silu = tmp.tile([P, TS], bf16, tag="silu")
                            nc.vector.tensor_mul(silu, g, sg)
                            cur_silu = silu
                        else:  # ffx_j
                            j = (fi - 3) // 2
                            fx = tmp.tile([P, TS], bf16, tag="fx")
                            nc.vector.tensor_mul(fx, pp, rstd_b[:, col])
                            hmul = nc.vector.tensor_mul(h_sb[j][:, col],
                                                        cur_silu, fx)
                            if ts == 1 and fi == NF - 1:
                                gate["i"] = hmul
                    # v -> token-major, into the v_aug tiles for this slice
                    for kt in range(ts * (TS // P), (ts + 1) * (TS // P)):
                        nc.sync.dma_start(
                            out=v_aug[kt][:, 0:64],
                            in_=kv_sb[64:128, kt * P:(kt + 1) * P],
                            transpose=True)

                # schedule: only stats(0) ahead of proj(0); later slices'
                # stats (and their row math / broadcasts / centering) hide
                # under the previous slice's projection
                emit_stats(0)
                emit_stats(1)
                emit_center(0)
                emit_proj(0)
                emit_stats(2)
                emit_center(1)
                emit_proj(1)
                emit_stats(3)
                emit_center(2)
                emit_proj(2)
                emit_center(3)
                emit_proj(3)
                # output-side weights: needed only ~190us in.  Explicitly
                # gated behind the end of proj slice 1 so the scheduler does
                # not hoist these (dependency-free) DMAs ahead of the x/wf
                # loads and halve the effective prologue load bandwidth.
                wao_sb = wp.tile([P, D], bf16)
                w_in = nc.gpsimd.dma_start(out=wao_sb, in_=wao_d[:, :])
                add_dep_helper(w_in.ins, gate["i"].ins,
                               reason="defer wao load")
                wfo_sb = []
                for k in range(NK):
                    t_ = wp.tile([P, D], bf16, name=f"wfo{k}")
                    w_in = nc.gpsimd.dma_start(out=t_,
                                               in_=wfo_d[k * P:(k + 1) * P, :])
                    add_dep_helper(w_in.ins, gate["i"].ins,
                                   reason="defer wfo load")
                    wfo_sb.append(t_)

            # xp closed: x/wf tiles are dead, reuse SBUF for attention tiles.
            # Attention pipeline over tsq-slots.  Head-0 sims run in PE rows
            # 0-63, head-1 sims concurrently in rows 64-127 (k replicated at
            # partitions 64-127, q head 1 already there).  AV matmuls of the
            # previous slot and y-chain matmuls interleave at ~exp rate so
            # the PE stays busy while ACT churns the exps.
            with (
                tc.tile_pool(name="esp", bufs=44) as esp,
                tc.tile_pool(name="atmp", bufs=2) as atmp,
                tc.tile_pool(name="yp", bufs=3) as yp,
                tc.tile_pool(name="yffp", bufs=1) as yffp,
            ):
                es_store = {}
                pavs = {}
                y_chains = []

                # Slice 0's output chains are split: the ff-only part runs
                # during attention slots 0-1 (when no other y work is
                # unlocked yet and the PE would otherwise wait on ACT exps),
                # accumulating to SBUF; the single attn matmul merges in
                # during evacuation once slice 0's attention output exists.
                yff_sb = [yffp.tile([P, TS], f32, name=f"yff{d}")
                          for d in range(NK)]

                def y_ff_chain_gen(d):
                    qcol = slice(0, TS)
                    py = ps.tile([P, TS], f32, tag="pp", bufs=5,
                                 name=f"pyf0_{d}")
                    for k in range(NK):
                        nc.tensor.matmul(
                            py, lhsT=wfo_sb[k][:, d * P:(d + 1) * P],
                            rhs=h_sb[k][:, qcol],
                            start=(k == 0), stop=(k == NK - 1))
                        yield
                    nc.vector.tensor_copy(yff_sb[d], py)

                def y_attn_chain_gen(d):
                    qcol = slice(0, TS)
                    pa = ps.tile([P, TS], f32, tag="pp", bufs=5,
                                 name=f"pya0_{d}")
                    nc.tensor.matmul(pa, lhsT=wao_sb[:, d * P:(d + 1) * P],
                                     rhs=out_sb[:, qcol],
                                     start=True, stop=True)
                    yield
                    y_sb = yp.tile([P, TS], f32, tag="ysb", name=f"ysba0_{d}")
                    nc.vector.scalar_tensor_tensor(
                        y_sb, pa, 1.0, yff_sb[d],
                        op0=mybir.AluOpType.mult,
                        op1=mybir.AluOpType.add)
                    nc.gpsimd.dma_start(out=yT_d[d * P:(d + 1) * P, qcol],
                                        in_=y_sb)

                def y_chain_gen(tsq, d):
                    qcol = slice(tsq * TS, (tsq + 1) * TS)
                    py = ps.tile([P, TS], f32, tag="pp", bufs=5,
                                 name=f"py{tsq}_{d}")
                    for k in range(NK):
                        nc.tensor.matmul(
                            py, lhsT=wfo_sb[k][:, d * P:(d + 1) * P],
                            rhs=h_sb[k][:, qcol],
                            start=(k == 0), stop=False)
                        yield
                    nc.tensor.matmul(
                        py, lhsT=wao_sb[:, d * P:(d + 1) * P],
                        rhs=out_sb[:, qcol], start=False, stop=True)
                    y_sb = yp.tile([P, TS], f32, tag="ysb",
                                   name=f"ysb{tsq}_{d}")
                    nc.vector.tensor_copy(y_sb, py)
                    nc.gpsimd.dma_start(out=yT_d[d * P:(d + 1) * P, qcol],
                                        in_=y_sb)

                def y_step(n):
                    done = 0
                    while done < n and y_chains:
                        try:
                            next(y_chains[0])
                        except StopIteration:
                            y_chains.pop(0)
                        done += 1

                def emit_sims_pair(tsq, kt):
                    qcol = slice(tsq * TS, (tsq + 1) * TS)
                    kcols = slice(kt * P, (kt + 1) * P)
                    for h in range(LH):
                        psim = ps.tile([P, TS], f32, tag="pp", bufs=5,
                                       name=f"psim{tsq}_{h}_{kt}")
                        if h == 0:
                            nc.tensor.matmul(psim, lhsT=kv_sb[0:64, kcols],
                                             rhs=q2_sb[0:64, qcol],
                                             start=True, stop=True)
                        else:
                            nc.tensor.matmul(psim, lhsT=k2_sb[64:128, kcols],
                                             rhs=q2_sb[64:128, qcol],
                                             start=True, stop=True)
                        es = esp.tile([P, TS], bf16, tag="es",
                                      name=f"es{tsq}_{h}_{kt}")
                        nc.scalar.activation(es, psim, AF.Exp, bias=zero_col)
                        es_store[(tsq, h, kt)] = es

                def av_mm(tsq, h, kt):
                    if kt == 0:
                        pavs[(tsq, h)] = ps.tile([P, TS], f32, tag="pav",
                                                 bufs=3, name=f"pav{tsq}_{h}")
                    nc.tensor.matmul(
                        pavs[(tsq, h)][0:65, :], lhsT=v_aug[kt][:, 0:65],
                        rhs=es_store.pop((tsq, h, kt)),
                        start=(kt == 0), stop=(kt == NKT - 1))

                def emit_av_epilogue(tsq, h):
                    b = tsq * LH + h
                    qcol = slice(tsq * TS, (tsq + 1) * TS)
                    pav = pavs.pop((tsq, h))
                    # denominator (partition 64) -> reciprocal (stays at
                    # partition 64) -> K=1 fp32 PE broadcast over 64 rows,
                    # reading the stationary+moving operands at partition 64
                    rec64 = atmp.tile([P, TS], mybir.dt.float32r,
                                      tag="rec64")
                    with nc.allow_low_precision(
                            reason="f32r broadcast operand; ~19-bit "
                                   "mantissa is plenty for 1/denom"):
                        nc.vector.reciprocal(rec64[64:65, :],
                                             pav[64:65, :])
                    pB = ps.tile([64, TS], f32, tag="pp", bufs=5,
                                 name=f"pB{b}")
                    nc.tensor.matmul(pB, lhsT=ones_hi[64:65, :],
                                     rhs=rec64[64:65, :],
                                     start=True, stop=True)
                    rb = atmp.tile([64, TS], f32, tag="rb")
                    nc.vector.tensor_copy(rb, pB)
                    if h == 0:
                        nc.vector.tensor_mul(out_sb[0:64, qcol],
                                             pav[0:64, :], rb)
                    else:
                        oh1 = atmp.tile([64, TS], bf16, tag="oh1")
                        nc.vector.tensor_mul(oh1, pav[0:64, :], rb)
                        nc.gpsimd.dma_start(out=out_sb[64:128, qcol], in_=oh1)
                        if tsq == 0:
                            y_chains.extend(y_attn_chain_gen(d)
                                            for d in range(NK))
                        else:
                            y_chains.extend(y_chain_gen(tsq, d)
                                            for d in range(NK))

                # slot 0: sims, with slice-0 ff chains as PE filler
                y_chains.extend(y_ff_chain_gen(d) for d in range(NK))
                for kt in range(NKT):
                    emit_sims_pair(0, kt)
                    y_step(2)
                # slots 1..NTS-1
                for b in range(1, NTS):
                    for kt in range(NKT):
                        emit_sims_pair(b, kt)
                        av_mm(b - 1, 0, kt)
                        if kt == NKT - 1:
                            emit_av_epilogue(b - 1, 0)
                        av_mm(b - 1, 1, kt)
                        y_step(3)
                    emit_av_epilogue(b - 1, 1)
                # final AV slot
                for kt in range(NKT):
                    av_mm(NTS - 1, 0, kt)
                    av_mm(NTS - 1, 1, kt)
                    y_step(3)
                emit_av_epilogue(NTS - 1, 0)
                emit_av_epilogue(NTS - 1, 1)
                y_step(1 << 30)

    nc.compile()
    return nc


def _get_nc():
    if "nc" not in _STATE:
        _STATE["nc"] = _build_nc()
    return _STATE["nc"]


def _prep_inputs(x, gamma, w_fused, w_attn_out, w_ff_out):
    """Host-side shard packing. Returns in_maps for the 8 cores."""
    x = np.asarray(x, dtype=np.float32)
    gamma = np.asarray(gamma, dtype=np.float32)
    w_fused = np.asarray(w_fused, dtype=np.float32)
    w_attn_out = np.asarray(w_attn_out, dtype=np.float32)
    w_ff_out = np.asarray(w_ff_out, dtype=np.float32)

    # fold gamma into w_fused rows; fold q scale into q columns
    wf = w_fused * gamma[:, None]
    wf = wf.copy()
    wf[:, :ATTN_INNER] *= DH ** -0.5

    q_blk = wf[:, :ATTN_INNER]
    k_blk = wf[:, ATTN_INNER:ATTN_INNER + DH]
    v_blk = wf[:, ATTN_INNER + DH:ATTN_INNER + 2 * DH]
    ffx_blk = wf[:, ATTN_INNER + 2 * DH:ATTN_INNER + 2 * DH + FF_INNER]
    gate_blk = wf[:, ATTN_INNER + 2 * DH + FF_INNER:]

    xT = [np.ascontiguousarray(x[b].T).astype(_BF16) for b in range(B)]

    in_maps = []
    for c in range(NCORES):
        b, s = divmod(c, TP)
        cols = [q_blk[:, P * s:P * s + P], k_blk, v_blk]
        for j in range(NK):
            cols.append(gate_blk[:, FF_SH * s + j * P: FF_SH * s + (j + 1) * P])
            cols.append(ffx_blk[:, FF_SH * s + j * P: FF_SH * s + (j + 1) * P])
        wf_c = np.concatenate(cols, axis=1).astype(_BF16)
        wao_c = np.ascontiguousarray(
            w_attn_out[P * s:P * s + P, :]).astype(_BF16)
        wfo_c = np.ascontiguousarray(
            w_ff_out[FF_SH * s:FF_SH * (s + 1), :]).astype(_BF16)
        in_maps.append({"xT": xT[b], "wf": wf_c, "wao": wao_c, "wfo": wfo_c})
    return in_maps


def kernel(x, gamma, w_fused, w_attn_out, w_ff_out):
    import time
    from concourse.bass_utils import run_bass_kernel_spmd

    nc = _get_nc()
    in_maps = _prep_inputs(x, gamma, w_fused, w_attn_out, w_ff_out)

    t0 = time.perf_counter()
    res = run_bass_kernel_spmd(nc, in_maps, core_ids=list(range(NCORES)))
    t1 = time.perf_counter()
    _STATE["last_wall_ns"] = (t1 - t0) * 1e9

    y = np.empty((B, N, D), dtype=np.float32)
    for b in range(B):
        acc = res.results[b * TP]["yT"].astype(np.float32)
        for s in range(1, TP):
            acc = acc + res.results[b * TP + s]["yT"]
        y[b] = acc.T
    return y


# revision 49
# speedup vs baseline: 1.0719x; 1.0295x over previous
"""Fused transformer block (LN + fused QKV/FF proj + MQA attention + SwiGLU FF)
on 8 TRN2 NeuronCores.

Sharding: hybrid DP2 x TP4.
  core c -> batch b = c//4, tensor-parallel shard s = c%4.
  Each core handles its batch's full 2048 tokens in feature-major layout:
    - q: 2 of 8 heads (cols 128*s .. 128*s+128 of the q block)
    - k/v: replicated (width 64 each)
    - ff: 1024 of 4096 cols of both ff_x and gate
    - attn_out / ff_out: matching row shards -> partial [1024, 2048] outputs
  Host sums the 4 partial outputs per batch (row-parallel reduction).

Device layout is feature-major (features on SBUF partitions, tokens on the
free dim) so every matmul contraction is over the partition dim.  gamma and
the q-scale (dim_head**-0.5) are folded into w_fused on the host; per-token
LayerNorm mu/rstd are computed on device via ones-vector matmuls, broadcast
across partitions through a DRAM bounce, mean-subtract applied in place on
x^T, and rstd folded into each projection's PSUM evacuation.
"""

import numpy as np
import ml_dtypes

# ---- problem shapes (hardcoded) ----
B, N, D = 2, 2048, 1024
DH = 64
HEADS = 8
ATTN_INNER = HEADS * DH          # 512
FF_INNER = 4 * D                 # 4096
T = N                            # tokens per core
P = 128
TS = 512
NTS = T // TS                    # 4
NK = D // P                      # 8
NCORES = 8
TP = 4
LH = HEADS // TP                 # 2 local heads
FF_SH = FF_INNER // TP           # 1024
FSH = LH * DH + 2 * DH + 2 * FF_SH   # 2304 packed proj cols per core
NF = FSH // P                    # 18
NKT = T // P                     # 16 key-token tiles

_BF16 = ml_dtypes.bfloat16

_STATE = {}


def _build_nc():
    from concourse import bacc
    import concourse.tile as tile
    from concourse.tile import add_dep_helper
    import concourse.mybir as mybir

    bf16 = mybir.dt.bfloat16
    f32 = mybir.dt.float32
    AF = mybir.ActivationFunctionType

    nc = bacc.Bacc("TRN2", target_bir_lowering=False, debug=False)

    xT_d = nc.dram_tensor("xT", [D, T], bf16, kind="ExternalInput")
    wf_d = nc.dram_tensor("wf", [D, FSH], bf16, kind="ExternalInput")
    wao_d = nc.dram_tensor("wao", [P, D], bf16, kind="ExternalInput")
    wfo_d = nc.dram_tensor("wfo", [D, D], bf16, kind="ExternalInput")
    yT_d = nc.dram_tensor("yT", [D, T], f32, kind="ExternalOutput")
    # DRAM bounce rows for partition-broadcast
    mu_d = nc.dram_tensor("mu_bounce", [1, T], bf16)
    rstd_d = nc.dram_tensor("rstd_bounce", [1, T], f32)

    with tile.TileContext(nc) as tc:
        with (
            tc.tile_pool(name="cp", bufs=1) as cp,
            tc.tile_pool(name="wp", bufs=1) as wp,
            tc.tile_pool(name="acts", bufs=1) as acts,
            tc.tile_pool(name="rows", bufs=1) as rows,
            tc.tile_pool(name="tmp", bufs=4) as tmp,
            tc.tile_pool(name="ps", bufs=1, space="PSUM") as ps,
        ):
            # ---- constants ----
            ones_col = cp.tile([P, 1], bf16)
            nc.vector.memset(ones_col, 1.0)
            ones_row_bf = cp.tile([1, P], bf16)
            nc.vector.memset(ones_row_bf, 1.0)
            f32r = mybir.dt.float32r
            ones_hi = cp.tile([P, 64], f32r)
            nc.vector.memset(ones_hi.bitcast(f32), 1.0)
            ones_row_r = cp.tile([1, P], f32r)
            nc.vector.memset(ones_row_r.bitcast(f32), 1.0)
            eps_t = cp.tile([1, 1], f32)
            nc.vector.memset(eps_t, 1e-5)
            zero_col = cp.tile([P, 1], f32)
            nc.vector.memset(zero_col, 0.0)
            # warm the ACT function tables before the bulk DMAs queue up:
            # lazy table loads otherwise serialize behind ~9MB of weight
            # traffic and stall the first LayerNorm square by ~8us
            warm_o = cp.tile([1, 4], f32)
            for wi, fn in enumerate((AF.Square, AF.Sqrt, AF.Sigmoid, AF.Exp)):
                nc.scalar.activation(warm_o[0:1, wi:wi + 1],
                                     eps_t, fn, bias=zero_col[0:1, :])

            # ---- persistent activations ----
            q2_sb = acts.tile([P, T], bf16)    # q both local heads, feature-major
            kv_sb = acts.tile([P, T], bf16)    # k rows 0-63, v rows 64-127
            k2_sb = acts.tile([P, T], bf16)    # k replica at partitions 64-127
                                               # (head-1 sim row-group packing)
            h_sb = [acts.tile([P, T], bf16, name=f"h{j}") for j in range(NK)]
            out_sb = acts.tile([P, T], bf16)   # attention out, both heads
            mu_b = acts.tile([P, T], bf16)     # mu broadcast
            rstd_b = acts.tile([P, T], f32)    # rstd broadcast
            v_aug = [acts.tile([P, 72], bf16, name=f"va{kt}") for kt in range(NKT)]
            for kt in range(NKT):
                nc.vector.memset(v_aug[kt][:, 64:65], 1.0)

            with tc.tile_pool(name="xp", bufs=1) as xp:
                # ---- load x^T, ts-chunked so stats/proj of slice 0 start
                # as early as possible ----
                # x in two column-halves and wf in column-quarters,
                # interleaved so that (a) slice-0/1 stats are gated on only
                # half of x and (b) the first proj chains are gated on only
                # the first quarter of wf -> the projection starts ~10us
                # earlier than with monolithic loads
                xt = [xp.tile([P, T], bf16, name=f"xt{k}") for k in range(NK)]
                wf_sb = [xp.tile([P, FSH], bf16, name=f"wf{k}")
                         for k in range(NK)]
                WQ = FSH // 4
                xh1 = slice(0, T // 2)
                xh2 = slice(T // 2, T)
                for k in range(NK):
                    nc.sync.dma_start(out=xt[k][:, xh1],
                                      in_=xT_d[k * P:(k + 1) * P, xh1])
                for q in range(2):
                    qc = slice(q * WQ, (q + 1) * WQ)
                    for k in range(NK):
                        nc.sync.dma_start(out=wf_sb[k][:, qc],
                                          in_=wf_d[k * P:(k + 1) * P, qc])
                for k in range(NK):
                    nc.sync.dma_start(out=xt[k][:, xh2],
                                      in_=xT_d[k * P:(k + 1) * P, xh2])
                for q in range(2, 4):
                    qc = slice(q * WQ, (q + 1) * WQ)
                    for k in range(NK):
                        nc.sync.dma_start(out=wf_sb[k][:, qc],
                                          in_=wf_d[k * P:(k + 1) * P, qc])
                # ---- LayerNorm statistics, per token-slice ----
                def emit_stats(ts):
                    col = slice(ts * TS, (ts + 1) * TS)
                    ps_s = ps.tile([1, TS], f32, tag="pp", bufs=5,
                                   name=f"ps_s{ts}")
                    for k in range(NK):
                        nc.tensor.matmul(ps_s, lhsT=ones_col,
                                         rhs=xt[k][:, col],
                                         start=(k == 0), stop=(k == NK - 1))
                    ps_s2 = ps.tile([1, TS], f32, tag="pp", bufs=5,
                                    name=f"ps_s2{ts}")
                    for k in range(NK):
                        x2t = tmp.tile([P, TS], bf16, tag="x2t")
                        nc.vector.tensor_mul(x2t, xt[k][:, col], xt[k][:, col])
                        nc.tensor.matmul(ps_s2, lhsT=ones_col, rhs=x2t,
                                         start=(k == 0), stop=(k == NK - 1))
                    # negvarD = (s^2)/D - s2 = -D*var ;  std = sqrt(-negvarD/D
                    # + eps) ;  mu(bf16) = s/D
                    ssq = rows.tile([1, TS], f32, tag="ssq")
                    nc.scalar.activation(ssq, ps_s, AF.Square,
                         bias=zero_col[0:1, :])
                    mu_bf_r = rows.tile([1, TS], bf16, tag="mu_bf_r")
                    nc.vector.tensor_scalar_mul(mu_bf_r, ps_s, 1.0 / D)
                    negvar = rows.tile([1, TS], f32, tag="negvar")
                    nc.vector.scalar_tensor_tensor(
                        negvar, ssq, 1.0 / D, ps_s2,
                        op0=mybir.AluOpType.mult,
                        op1=mybir.AluOpType.subtract)
                    std = rows.tile([1, TS], f32, tag="std")
                    nc.scalar.activation(std, negvar, AF.Sqrt, bias=eps_t,
                                         scale=-1.0 / D)
                    rstd_r = rows.tile([1, TS],
                                       f32r if ts == 0 else f32,
                                       tag="rstd_r")
                    with nc.allow_low_precision(
                            reason="f32r broadcast operand; ~19-bit "
                                   "mantissa is plenty for rstd"):
                        nc.vector.reciprocal(rstd_r, std)
                    if ts == 0:
                        # slice 0 gates the whole projection phase: broadcast
                        # via K=1 PE matmuls (DMA-free; the DRAM-bounce path
                        # would queue behind the bulk weight loads)
                        pmu = ps.tile([P, TS], f32, tag="pp", bufs=5,
                                      name="pmu0")
                        nc.tensor.matmul(pmu, lhsT=ones_row_bf[0:1, :],
                                         rhs=mu_bf_r, start=True, stop=True)
                        nc.vector.tensor_copy(mu_b[:, col], pmu)
                        prs = ps.tile([P, TS], f32, tag="pp", bufs=5,
                                      name="prs0")
                        nc.tensor.matmul(prs, lhsT=ones_row_r[0:1, :],
                                         rhs=rstd_r,
                                         start=True, stop=True)
                        nc.vector.tensor_copy(rstd_b[:, col], prs)
                    else:
                        # bounce rows through DRAM, broadcast to 128
                        # partitions (gpsimd SWDGE queue; lands during the
                        # previous slice's projection)
                        nc.gpsimd.dma_start(out=mu_d[0:1, col], in_=mu_bf_r)
                        nc.gpsimd.dma_start(out=rstd_d[0:1, col], in_=rstd_r)
                        nc.gpsimd.dma_start(
                            out=mu_b[:, col],
                            in_=mu_d[0:1, col].partition_broadcast(P))
                        nc.gpsimd.dma_start(
                            out=rstd_b[:, col],
                            in_=rstd_d[0:1, col].partition_broadcast(P))

                gate = {}

                def emit_center(ts):
                    # center x in place (emitted separately: this blocks DVE
                    # on the mu broadcast, so it must come after all stats
                    # squares that later PE chains depend on)
                    col = slice(ts * TS, (ts + 1) * TS)
                    for k in range(NK):
                        nc.vector.tensor_sub(xt[k][:, col], xt[k][:, col],
                                             mu_b[:, col])

                # ---- fused projection ----
                # packed col order: [q(128) | kv(128) | (gate_j, ffx_j) x 8]
                def emit_proj(ts):
                    col = slice(ts * TS, (ts + 1) * TS)
                    cur_silu = None
                    for fi in range(NF):
                        pp = ps.tile([P, TS], f32, tag="pp", bufs=5,
                                     name=f"pp{ts}_{fi}")
                        for k in range(NK):
                            nc.tensor.matmul(
                                pp,
                                lhsT=wf_sb[k][:, fi * P:(fi + 1) * P],
                                rhs=xt[k][:, col],
                                start=(k == 0), stop=(k == NK - 1))
                        if fi == 0:
                            nc.vector.tensor_mul(q2_sb[:, col], pp,
                                                 rstd_b[:, col])
                        elif fi == 1:
                            nc.vector.tensor_mul(kv_sb[:, col], pp,
                                                 rstd_b[:, col])
                            # replicate k rows to partitions 64-127 so head-1
                            # sims run in the upper PE row group
                            nc.sync.dma_start(out=k2_sb[64:128, col],
                                               in_=kv_sb[0:64, col])
                        elif fi % 2 == 0:  # gate_j
                            g = tmp.tile([P, TS], bf16, tag="g")
                            nc.vector.tensor_mul(g, pp, rstd_b[:, col])
                            sg = tmp.tile([P, TS], bf16, tag="sg")
                            nc.scalar.activation(sg, g, AF.Sigmoid, bias=zero_col)
                            silu = tmp.tile([P, TS], bf16, tag="silu")
                            nc.vector.tensor_mul(silu, g, sg)
                            cur_silu = silu
                        else:  # ffx_j
                            j = (fi - 3) // 2
                            fx = tmp.tile([P, TS], bf16, tag="fx")
                            nc.vector.tensor_mul(fx, pp, rstd_b[:, col])
                            hmul = nc.vector.tensor_mul(h_sb[j][:, col],
                                                        cur_silu, fx)
                            if ts == 1 and fi == NF - 1:
                                gate["i"] = hmul
                    # v -> token-major, into the v_aug tiles for this slice
                    for kt in range(ts * (TS // P), (ts + 1) * (TS // P)):
                        nc.sync.dma_start(
                            out=v_aug[kt][:, 0:64],
                            in_=kv_sb[64:128, kt * P:(kt + 1) * P],
                            transpose=True)

                # schedule: only stats(0) ahead of proj(0); later slices'
                # stats (and their row math / broadcasts / centering) hide
                # under the previous slice's projection
                emit_stats(0)
                emit_stats(1)
                emit_center(0)
                emit_proj(0)
                emit_stats(2)
                emit_center(1)
                emit_proj(1)
                emit_stats(3)
                emit_center(2)
                emit_proj(2)
                emit_center(3)
                emit_proj(3)
                # output-side weights: needed only ~190us in.  Explicitly
                # gated behind the end of proj slice 1 so the scheduler does
                # not hoist these (dependency-free) DMAs ahead of the x/wf
                # loads and halve the effective prologue load bandwidth.
                wao_sb = wp.tile([P, D], bf16)
                w_in = nc.gpsimd.dma_start(out=wao_sb, in_=wao_d[:, :])
                add_dep_helper(w_in.ins, gate["i"].ins,
                               reason="defer wao load")
                wfo_sb = []
                for k in range(NK):
                    t_ = wp.tile([P, D], bf16, name=f"wfo{k}")
                    w_in = nc.gpsimd.dma_start(out=t_,
                                               in_=wfo_d[k * P:(k + 1) * P, :])
                    add_dep_helper(w_in.ins, gate["i"].ins,
                                   reason="defer wfo load")
                    wfo_sb.append(t_)

            # xp closed: x/wf tiles are dead, reuse SBUF for attention tiles.
            # Attention pipeline over tsq-slots.  Head-0 sims run in PE rows
            # 0-63, head-1 sims concurrently in rows 64-127 (k replicated at
            # partitions 64-127, q head 1 already there).  AV matmuls of the
            # previous slot and y-chain matmuls interleave at ~exp rate so
            # the PE stays busy while ACT churns the exps.
            with (
                tc.tile_pool(name="esp", bufs=44) as esp,
                tc.tile_pool(name="atmp", bufs=3) as atmp,
                tc.tile_pool(name="yp", bufs=4) as yp,
                tc.tile_pool(name="yffp", bufs=1) as yffp,
            ):
                es_store = {}
                pavs = {}
                y_chains = []

                # Slice 0's output chains are split: the ff-only part runs
                # during attention slots 0-1 (when no other y work is
                # unlocked yet and the PE would otherwise wait on ACT exps),
                # accumulating to SBUF; the single attn matmul merges in
                # during evacuation once slice 0's attention output exists.
                yff_sb = [yffp.tile([P, TS], f32, name=f"yff{d}")
                          for d in range(NK)]

                def y_ff_chain_gen(d):
                    qcol = slice(0, TS)
                    py = ps.tile([P, TS], f32, tag="pp", bufs=5,
                                 name=f"pyf0_{d}")
                    for k in range(NK):
                        nc.tensor.matmul(
                            py, lhsT=wfo_sb[k][:, d * P:(d + 1) * P],
                            rhs=h_sb[k][:, qcol],
                            start=(k == 0), stop=(k == NK - 1))
                        yield
                    nc.vector.tensor_copy(yff_sb[d], py)

                def y_attn_chain_gen(d):
                    qcol = slice(0, TS)
                    pa = ps.tile([P, TS], f32, tag="pp", bufs=5,
                                 name=f"pya0_{d}")
                    nc.tensor.matmul(pa, lhsT=wao_sb[:, d * P:(d + 1) * P],
                                     rhs=out_sb[:, qcol],
                                     start=True, stop=True)
                    yield
                    y_sb = yp.tile([P, TS], f32, tag="ysb", name=f"ysba0_{d}")
                    nc.vector.scalar_tensor_tensor(
                        y_sb, pa, 1.0, yff_sb[d],
                        op0=mybir.AluOpType.mult,
                        op1=mybir.AluOpType.add)
                    nc.gpsimd.dma_start(out=yT_d[d * P:(d + 1) * P, qcol],
                                        in_=y_sb)

                def y_chain_gen(tsq, d):
                    qcol = slice(tsq * TS, (tsq + 1) * TS)
                    py = ps.tile([P, TS], f32, tag="pp", bufs=5,
                                 name=f"py{tsq}_{d}")
                    for k in range(NK):
                        nc.tensor.matmul(
                            py, lhsT=wfo_sb[k][:, d * P:(d + 1) * P],
                            rhs=h_sb[k][:, qcol],
                            start=(k == 0), stop=False)
                        yield
                    nc.tensor.matmul(
                        py, lhsT=wao_sb[:, d * P:(d + 1) * P],
                        rhs=out_sb[:, qcol], start=False, stop=True)
                    y_sb = yp.tile([P, TS], f32, tag="ysb",
                                   name=f"ysb{tsq}_{d}")
                    nc.vector.tensor_copy(y_sb, py)
                    nc.gpsimd.dma_start(out=yT_d[d * P:(d + 1) * P, qcol],
                                        in_=y_sb)

                def y_step(n):
                    done = 0
                    while done < n and y_chains:
                        try:
                            next(y_chains[0])
                        except StopIteration:
                            y_chains.pop(0)
                        done += 1

                def emit_sims_pair(tsq, kt):
                    qcol = slice(tsq * TS, (tsq + 1) * TS)
                    kcols = slice(kt * P, (kt + 1) * P)
                    for h in range(LH):
                        psim = ps.tile([P, TS], f32, tag="pp", bufs=5,
                                       name=f"psim{tsq}_{h}_{kt}")
                        if h == 0:
                            nc.tensor.matmul(psim, lhsT=kv_sb[0:64, kcols],
                                             rhs=q2_sb[0:64, qcol],
                                             start=True, stop=True)
                        else:
                            nc.tensor.matmul(psim, lhsT=k2_sb[64:128, kcols],
                                             rhs=q2_sb[64:128, qcol],
                                             start=True, stop=True)
                        es = esp.tile([P, TS], bf16, tag="es",
                                      name=f"es{tsq}_{h}_{kt}")
                        nc.scalar.activation(es, psim, AF.Exp, bias=zero_col)
                        es_store[(tsq, h, kt)] = es

                def av_mm(tsq, h, kt):
                    if kt == 0:
                        pavs[(tsq, h)] = ps.tile([P, TS], f32, tag="pav",
                                                 bufs=3, name=f"pav{tsq}_{h}")
                    nc.tensor.matmul(
                        pavs[(tsq, h)][0:65, :], lhsT=v_aug[kt][:, 0:65],
                        rhs=es_store.pop((tsq, h, kt)),
                        start=(kt == 0), stop=(kt == NKT - 1))

                def emit_av_epilogue(tsq, h):
                    b = tsq * LH + h
                    qcol = slice(tsq * TS, (tsq + 1) * TS)
                    pav = pavs.pop((tsq, h))
                    # denominator (partition 64) -> reciprocal (stays at
                    # partition 64) -> K=1 fp32 PE broadcast over 64 rows,
                    # reading the stationary+moving operands at partition 64
                    rec64 = atmp.tile([P, TS], mybir.dt.float32r,
                                      tag="rec64")
                    with nc.allow_low_precision(
                            reason="f32r broadcast operand; ~19-bit "
                                   "mantissa is plenty for 1/denom"):
                        nc.vector.reciprocal(rec64[64:65, :],
                                             pav[64:65, :])
                    pB = ps.tile([64, TS], f32, tag="pp", bufs=5,
                                 name=f"pB{b}")
                    nc.tensor.matmul(pB, lhsT=ones_hi[64:65, :],
                                     rhs=rec64[64:65, :],
                                     start=True, stop=True)
                    rb = atmp.tile([64, TS], f32, tag="rb")
                    nc.vector.tensor_copy(rb, pB)
                    if h == 0:
                        nc.vector.tensor_mul(out_sb[0:64, qcol],
                                             pav[0:64, :], rb)
                    else:
                        oh1 = atmp.tile([64, TS], bf16, tag="oh1")
                        nc.vector.tensor_mul(oh1, pav[0:64, :], rb)
                        nc.gpsimd.dma_start(out=out_sb[64:128, qcol], in_=oh1)
                        if tsq == 0:
                            y_chains.extend(y_attn_chain_gen(d)
                                            for d in range(NK))
                        else:
                            y_chains.extend(y_chain_gen(tsq, d)
                                            for d in range(NK))

                # slot 0: sims, with slice-0 ff chains as PE filler
                y_chains.extend(y_ff_chain_gen(d) for d in range(NK))
                for kt in range(NKT):
                    emit_sims_pair(0, kt)
                    y_step(2)
                # slots 1..NTS-1
                for b in range(1, NTS):
                    for kt in range(NKT):
                        emit_sims_pair(b, kt)
                        av_mm(b - 1, 0, kt)
                        if kt == NKT - 1:
                            emit_av_epilogue(b - 1, 0)
                        av_mm(b - 1, 1, kt)
                        y_step(3)
                    emit_av_epilogue(b - 1, 1)
                # final AV slot
                for kt in range(NKT):
                    av_mm(NTS - 1, 0, kt)
                    av_mm(NTS - 1, 1, kt)
                    y_step(3)
                emit_av_epilogue(NTS - 1, 0)
                emit_av_epilogue(NTS - 1, 1)
                y_step(1 << 30)

    nc.compile()
    return nc


def _get_nc():
    if "nc" not in _STATE:
        _STATE["nc"] = _build_nc()
    return _STATE["nc"]


def _prep_inputs(x, gamma, w_fused, w_attn_out, w_ff_out):
    """Host-side shard packing. Returns in_maps for the 8 cores."""
    x = np.asarray(x, dtype=np.float32)
    gamma = np.asarray(gamma, dtype=np.float32)
    w_fused = np.asarray(w_fused, dtype=np.float32)
    w_attn_out = np.asarray(w_attn_out, dtype=np.float32)
    w_ff_out = np.asarray(w_ff_out, dtype=np.float32)

    # fold gamma into w_fused rows; fold q scale into q columns
    wf = w_fused * gamma[:, None]
    wf = wf.copy()
    wf[:, :ATTN_INNER] *= DH ** -0.5

    q_blk = wf[:, :ATTN_INNER]
    k_blk = wf[:, ATTN_INNER:ATTN_INNER + DH]
    v_blk = wf[:, ATTN_INNER + DH:ATTN_INNER + 2 * DH]
    ffx_blk = wf[:, ATTN_INNER + 2 * DH:ATTN_INNER + 2 * DH + FF_INNER]
    gate_blk = wf[:, ATTN_INNER + 2 * DH + FF_INNER:]

    xT = [np.ascontiguousarray(x[b].T).astype(_BF16) for b in range(B)]

    in_maps = []
    for c in range(NCORES):
        b, s = divmod(c, TP)
        cols = [q_blk[:, P * s:P * s + P], k_blk, v_blk]
        for j in range(NK):
            cols.append(gate_blk[:, FF_SH * s + j * P: FF_SH * s + (j + 1) * P])
            cols.append(ffx_blk[:, FF_SH * s + j * P: FF_SH * s + (j + 1) * P])
        wf_c = np.concatenate(cols, axis=1).astype(_BF16)
        wao_c = np.ascontiguousarray(
            w_attn_out[P * s:P * s + P, :]).astype(_BF16)
        wfo_c = np.ascontiguousarray(
            w_ff_out[FF_SH * s:FF_SH * (s + 1), :]).astype(_BF16)
        in_maps.append({"xT": xT[b], "wf": wf_c, "wao": wao_c, "wfo": wfo_c})
    return in_maps


def kernel(x, gamma, w_fused, w_attn_out, w_ff_out):
    import time
    from concourse.bass_utils import run_bass_kernel_spmd

    nc = _get_nc()
    in_maps = _prep_inputs(x, gamma, w_fused, w_attn_out, w_ff_out)

    t0 = time.perf_counter()
    res = run_bass_kernel_spmd(nc, in_maps, core_ids=list(range(NCORES)))
    t1 = time.perf_counter()
    _STATE["last_wall_ns"] = (t1 - t0) * 1e9

    y = np.empty((B, N, D), dtype=np.float32)
    for b in range(B):
        acc = res.results[b * TP]["yT"].astype(np.float32)
        for s in range(1, TP):
            acc = acc + res.results[b * TP + s]["yT"]
        y[b] = acc.T
    return y


# revision 52
# speedup vs baseline: 1.0902x; 1.0171x over previous
"""Fused transformer block (LN + fused QKV/FF proj + MQA attention + SwiGLU FF)
on 8 TRN2 NeuronCores.

Sharding: hybrid DP2 x TP4.
  core c -> batch b = c//4, tensor-parallel shard s = c%4.
  Each core handles its batch's full 2048 tokens in feature-major layout:
    - q: 2 of 8 heads (cols 128*s .. 128*s+128 of the q block)
    - k/v: replicated (width 64 each)
    - ff: 1024 of 4096 cols of both ff_x and gate
    - attn_out / ff_out: matching row shards -> partial [1024, 2048] outputs
  Host sums the 4 partial outputs per batch (row-parallel reduction).

Device layout is feature-major (features on SBUF partitions, tokens on the
free dim) so every matmul contraction is over the partition dim.  gamma and
the q-scale (dim_head**-0.5) are folded into w_fused on the host; per-token
LayerNorm mu/rstd are computed on device via ones-vector matmuls, broadcast
across partitions through a DRAM bounce, mean-subtract applied in place on
x^T, and rstd folded into each projection's PSUM evacuation.
"""

import numpy as np
import ml_dtypes

# ---- problem shapes (hardcoded) ----
B, N, D = 2, 2048, 1024
DH = 64
HEADS = 8
ATTN_INNER = HEADS * DH          # 512
FF_INNER = 4 * D                 # 4096
T = N                            # tokens per core
P = 128
TS = 512
NTS = T // TS                    # 4
NK = D // P                      # 8
NCORES = 8
TP = 4
LH = HEADS // TP                 # 2 local heads
FF_SH = FF_INNER // TP           # 1024
FSH = LH * DH + 2 * DH + 2 * FF_SH   # 2304 packed proj cols per core
NF = FSH // P                    # 18
NKT = T // P                     # 16 key-token tiles

_BF16 = ml_dtypes.bfloat16

_STATE = {}


def _build_nc():
    from concourse import bacc
    import concourse.tile as tile
    from concourse.tile import add_dep_helper
    import concourse.mybir as mybir

    bf16 = mybir.dt.bfloat16
    f32 = mybir.dt.float32
    AF = mybir.ActivationFunctionType

    nc = bacc.Bacc("TRN2", target_bir_lowering=False, debug=False)

    xT_d = nc.dram_tensor("xT", [D, T], bf16, kind="ExternalInput")
    wf_d = nc.dram_tensor("wf", [D, FSH], bf16, kind="ExternalInput")
    wao_d = nc.dram_tensor("wao", [P, D], bf16, kind="ExternalInput")
    wfo_d = nc.dram_tensor("wfo", [D, D], bf16, kind="ExternalInput")
    yT_d = nc.dram_tensor("yT", [D, T], f32, kind="ExternalOutput")
    # DRAM bounce rows for partition-broadcast
    mu_d = nc.dram_tensor("mu_bounce", [1, T], bf16)
    rstd_d = nc.dram_tensor("rstd_bounce", [1, T], f32)

    with tile.TileContext(nc) as tc:
        with (
            tc.tile_pool(name="cp", bufs=1) as cp,
            tc.tile_pool(name="wp", bufs=1) as wp,
            tc.tile_pool(name="acts", bufs=1) as acts,
            tc.tile_pool(name="rows", bufs=1) as rows,
            tc.tile_pool(name="tmp", bufs=4) as tmp,
            tc.tile_pool(name="ps", bufs=1, space="PSUM") as ps,
        ):
            # ---- constants ----
            ones_col = cp.tile([P, 1], bf16)
            nc.vector.memset(ones_col, 1.0)
            ones_row_bf = cp.tile([1, P], bf16)
            nc.vector.memset(ones_row_bf, 1.0)
            f32r = mybir.dt.float32r
            ones_hi = cp.tile([P, 64], f32r)
            nc.vector.memset(ones_hi.bitcast(f32), 1.0)
            ones_row_r = cp.tile([1, P], f32r)
            nc.vector.memset(ones_row_r.bitcast(f32), 1.0)
            eps_t = cp.tile([1, 1], f32)
            nc.vector.memset(eps_t, 1e-5)
            zero_col = cp.tile([P, 1], f32)
            nc.vector.memset(zero_col, 0.0)
            # warm the ACT function tables before the bulk DMAs queue up:
            # lazy table loads otherwise serialize behind ~9MB of weight
            # traffic and stall the first LayerNorm square by ~8us
            warm_o = cp.tile([1, 4], f32)
            for wi, fn in enumerate((AF.Square, AF.Sqrt, AF.Sigmoid, AF.Exp)):
                nc.scalar.activation(warm_o[0:1, wi:wi + 1],
                                     eps_t, fn, bias=zero_col[0:1, :])

            # ---- persistent activations ----
            q2_sb = acts.tile([P, T], bf16)    # q both local heads, feature-major
            kv_sb = acts.tile([P, T], bf16)    # k rows 0-63, v rows 64-127
            k2_sb = acts.tile([P, T], bf16)    # k replica at partitions 64-127
                                               # (head-1 sim row-group packing)
            h_sb = [acts.tile([P, T], bf16, name=f"h{j}") for j in range(NK)]
            out_sb = acts.tile([P, T], bf16)   # attention out, both heads
            mu_b = acts.tile([P, T], bf16)     # mu broadcast
            rstd_b = acts.tile([P, T], f32)    # rstd broadcast
            v_aug = [acts.tile([P, 72], bf16, name=f"va{kt}") for kt in range(NKT)]
            for kt in range(NKT):
                nc.vector.memset(v_aug[kt][:, 64:65], 1.0)

            with tc.tile_pool(name="xp", bufs=1) as xp:
                # ---- load x^T, ts-chunked so stats/proj of slice 0 start
                # as early as possible ----
                # x in two column-halves and wf in column-quarters,
                # interleaved so that (a) slice-0/1 stats are gated on only
                # half of x and (b) the first proj chains are gated on only
                # the first quarter of wf -> the projection starts ~10us
                # earlier than with monolithic loads
                xt = [xp.tile([P, T], bf16, name=f"xt{k}") for k in range(NK)]
                wf_sb = [xp.tile([P, FSH], bf16, name=f"wf{k}")
                         for k in range(NK)]
                WQ = FSH // 4
                xh1 = slice(0, T // 2)
                xh2 = slice(T // 2, T)
                for k in range(NK):
                    nc.sync.dma_start(out=xt[k][:, xh1],
                                      in_=xT_d[k * P:(k + 1) * P, xh1])
                for q in range(2):
                    qc = slice(q * WQ, (q + 1) * WQ)
                    for k in range(NK):
                        nc.sync.dma_start(out=wf_sb[k][:, qc],
                                          in_=wf_d[k * P:(k + 1) * P, qc])
                for k in range(NK):
                    nc.sync.dma_start(out=xt[k][:, xh2],
                                      in_=xT_d[k * P:(k + 1) * P, xh2])
                for q in range(2, 4):
                    qc = slice(q * WQ, (q + 1) * WQ)
                    for k in range(NK):
                        nc.sync.dma_start(out=wf_sb[k][:, qc],
                                          in_=wf_d[k * P:(k + 1) * P, qc])
                # ---- LayerNorm statistics, per token-slice ----
                def emit_stats(ts):
                    col = slice(ts * TS, (ts + 1) * TS)
                    ps_s = ps.tile([1, TS], f32, tag="pp", bufs=5,
                                   name=f"ps_s{ts}")
                    for k in range(NK):
                        nc.tensor.matmul(ps_s, lhsT=ones_col,
                                         rhs=xt[k][:, col],
                                         start=(k == 0), stop=(k == NK - 1))
                    ps_s2 = ps.tile([1, TS], f32, tag="pp", bufs=5,
                                    name=f"ps_s2{ts}")
                    for k in range(NK):
                        x2t = tmp.tile([P, TS], bf16, tag="x2t")
                        nc.vector.tensor_mul(x2t, xt[k][:, col], xt[k][:, col])
                        nc.tensor.matmul(ps_s2, lhsT=ones_col, rhs=x2t,
                                         start=(k == 0), stop=(k == NK - 1))
                    # negvarD = (s^2)/D - s2 = -D*var ;  std = sqrt(-negvarD/D
                    # + eps) ;  mu(bf16) = s/D
                    ssq = rows.tile([1, TS], f32, tag="ssq")
                    nc.scalar.activation(ssq, ps_s, AF.Square,
                         bias=zero_col[0:1, :])
                    mu_bf_r = rows.tile([1, TS], bf16, tag="mu_bf_r")
                    nc.vector.tensor_scalar_mul(mu_bf_r, ps_s, 1.0 / D)
                    negvar = rows.tile([1, TS], f32, tag="negvar")
                    nc.vector.scalar_tensor_tensor(
                        negvar, ssq, 1.0 / D, ps_s2,
                        op0=mybir.AluOpType.mult,
                        op1=mybir.AluOpType.subtract)
                    std = rows.tile([1, TS], f32, tag="std")
                    nc.scalar.activation(std, negvar, AF.Sqrt, bias=eps_t,
                                         scale=-1.0 / D)
                    rstd_r = rows.tile([1, TS],
                                       f32r if ts == 0 else f32,
                                       tag="rstd_r")
                    with nc.allow_low_precision(
                            reason="f32r broadcast operand; ~19-bit "
                                   "mantissa is plenty for rstd"):
                        nc.vector.reciprocal(rstd_r, std)
                    if ts == 0:
                        # slice 0 gates the whole projection phase: broadcast
                        # via K=1 PE matmuls (DMA-free; the DRAM-bounce path
                        # would queue behind the bulk weight loads)
                        pmu = ps.tile([P, TS], f32, tag="pp", bufs=5,
                                      name="pmu0")
                        nc.tensor.matmul(pmu, lhsT=ones_row_bf[0:1, :],
                                         rhs=mu_bf_r, start=True, stop=True)
                        nc.vector.tensor_copy(mu_b[:, col], pmu)
                        prs = ps.tile([P, TS], f32, tag="pp", bufs=5,
                                      name="prs0")
                        nc.tensor.matmul(prs, lhsT=ones_row_r[0:1, :],
                                         rhs=rstd_r,
                                         start=True, stop=True)
                        nc.vector.tensor_copy(rstd_b[:, col], prs)
                    else:
                        # bounce rows through DRAM, broadcast to 128
                        # partitions (gpsimd SWDGE queue; lands during the
                        # previous slice's projection)
                        nc.gpsimd.dma_start(out=mu_d[0:1, col], in_=mu_bf_r)
                        nc.gpsimd.dma_start(out=rstd_d[0:1, col], in_=rstd_r)
                        nc.gpsimd.dma_start(
                            out=mu_b[:, col],
                            in_=mu_d[0:1, col].partition_broadcast(P))
                        nc.gpsimd.dma_start(
                            out=rstd_b[:, col],
                            in_=rstd_d[0:1, col].partition_broadcast(P))

                gate = {}

                def emit_center(ts):
                    # center x in place (emitted separately: this blocks DVE
                    # on the mu broadcast, so it must come after all stats
                    # squares that later PE chains depend on)
                    col = slice(ts * TS, (ts + 1) * TS)
                    for k in range(NK):
                        nc.vector.tensor_sub(xt[k][:, col], xt[k][:, col],
                                             mu_b[:, col])

                # ---- fused projection ----
                # packed col order: [q(128) | kv(128) | (gate_j, ffx_j) x 8]
                def emit_proj(ts):
                    col = slice(ts * TS, (ts + 1) * TS)
                    cur_silu = None
                    for fi in range(NF):
                        pp = ps.tile([P, TS], f32, tag="pp", bufs=5,
                                     name=f"pp{ts}_{fi}")
                        for k in range(NK):
                            nc.tensor.matmul(
                                pp,
                                lhsT=wf_sb[k][:, fi * P:(fi + 1) * P],
                                rhs=xt[k][:, col],
                                start=(k == 0), stop=(k == NK - 1))
                        if fi == 0:
                            nc.vector.tensor_mul(q2_sb[:, col], pp,
                                                 rstd_b[:, col])
                        elif fi == 1:
                            nc.vector.tensor_mul(kv_sb[:, col], pp,
                                                 rstd_b[:, col])
                            # replicate k rows to partitions 64-127 so head-1
                            # sims run in the upper PE row group
                            nc.sync.dma_start(out=k2_sb[64:128, col],
                                               in_=kv_sb[0:64, col])
                        elif fi % 2 == 0:  # gate_j
                            g = tmp.tile([P, TS], bf16, tag="g")
                            nc.vector.tensor_mul(g, pp, rstd_b[:, col])
                            sg = tmp.tile([P, TS], bf16, tag="sg")
                            nc.scalar.activation(sg, g, AF.Sigmoid, bias=zero_col)
                            silu = tmp.tile([P, TS], bf16, tag="silu")
                            nc.vector.tensor_mul(silu, g, sg)
                            cur_silu = silu
                        else:  # ffx_j
                            j = (fi - 3) // 2
                            fx = tmp.tile([P, TS], bf16, tag="fx")
                            nc.vector.tensor_mul(fx, pp, rstd_b[:, col])
                            hmul = nc.vector.tensor_mul(h_sb[j][:, col],
                                                        cur_silu, fx)
                            if ts == 1 and fi == NF - 1:
                                gate["i"] = hmul
                    # v -> token-major, into the v_aug tiles for this slice
                    for kt in range(ts * (TS // P), (ts + 1) * (TS // P)):
                        nc.sync.dma_start(
                            out=v_aug[kt][:, 0:64],
                            in_=kv_sb[64:128, kt * P:(kt + 1) * P],
                            transpose=True)

                # schedule: only stats(0) ahead of proj(0); later slices'
                # stats (and their row math / broadcasts / centering) hide
                # under the previous slice's projection
                emit_stats(0)
                emit_stats(1)
                emit_center(0)
                emit_proj(0)
                emit_stats(2)
                emit_center(1)
                emit_proj(1)
                emit_stats(3)
                emit_center(2)
                emit_proj(2)
                emit_center(3)
                emit_proj(3)
                # output-side weights: needed only ~190us in.  Explicitly
                # gated behind the end of proj slice 1 so the scheduler does
                # not hoist these (dependency-free) DMAs ahead of the x/wf
                # loads and halve the effective prologue load bandwidth.
                wao_sb = wp.tile([P, D], bf16)
                w_in = nc.gpsimd.dma_start(out=wao_sb, in_=wao_d[:, :])
                add_dep_helper(w_in.ins, gate["i"].ins,
                               reason="defer wao load")
                wfo_sb = []
                for k in range(NK):
                    t_ = wp.tile([P, D], bf16, name=f"wfo{k}")
                    w_in = nc.gpsimd.dma_start(out=t_,
                                               in_=wfo_d[k * P:(k + 1) * P, :])
                    add_dep_helper(w_in.ins, gate["i"].ins,
                                   reason="defer wfo load")
                    wfo_sb.append(t_)

            # xp closed: x/wf tiles are dead, reuse SBUF for attention tiles.
            # Attention pipeline over tsq-slots.  Head-0 sims run in PE rows
            # 0-63, head-1 sims concurrently in rows 64-127 (k replicated at
            # partitions 64-127, q head 1 already there).  AV matmuls of the
            # previous slot and y-chain matmuls interleave at ~exp rate so
            # the PE stays busy while ACT churns the exps.
            with (
                tc.tile_pool(name="esp", bufs=44) as esp,
                tc.tile_pool(name="atmp", bufs=3) as atmp,
                tc.tile_pool(name="yp", bufs=4) as yp,
                tc.tile_pool(name="yffp", bufs=1) as yffp,
            ):
                es_store = {}
                pavs = {}
                y_chains = []

                # Slice 0's output chains are split: the ff-only part runs
                # during attention slots 0-1 (when no other y work is
                # unlocked yet and the PE would otherwise wait on ACT exps),
                # accumulating to SBUF; the single attn matmul merges in
                # during evacuation once slice 0's attention output exists.
                yff_sb = [yffp.tile([P, TS], f32, name=f"yff{d}")
                          for d in range(NK)]

                def y_ff_chain_gen(tsq, d):
                    qcol = slice(tsq * TS, (tsq + 1) * TS)
                    py = ps.tile([P, TS], f32, tag="pp", bufs=5,
                                 name=f"pyf{tsq}_{d}")
                    for k in range(NK):
                        nc.tensor.matmul(
                            py, lhsT=wfo_sb[k][:, d * P:(d + 1) * P],
                            rhs=h_sb[k][:, qcol],
                            start=(k == 0), stop=(k == NK - 1))
                        yield
                    nc.vector.tensor_copy(yff_sb[d], py)

                def y_attn_chain_gen(tsq, d):
                    qcol = slice(tsq * TS, (tsq + 1) * TS)
                    pa = ps.tile([P, TS], f32, tag="pp", bufs=5,
                                 name=f"pya{tsq}_{d}")
                    nc.tensor.matmul(pa, lhsT=wao_sb[:, d * P:(d + 1) * P],
                                     rhs=out_sb[:, qcol],
                                     start=True, stop=True)
                    yield
                    y_sb = yp.tile([P, TS], f32, tag="ysb",
                                   name=f"ysba{tsq}_{d}")
                    nc.vector.scalar_tensor_tensor(
                        y_sb, pa, 1.0, yff_sb[d],
                        op0=mybir.AluOpType.mult,
                        op1=mybir.AluOpType.add)
                    nc.gpsimd.dma_start(out=yT_d[d * P:(d + 1) * P, qcol],
                                        in_=y_sb)

                def y_chain_gen(tsq, d):
                    qcol = slice(tsq * TS, (tsq + 1) * TS)
                    py = ps.tile([P, TS], f32, tag="pp", bufs=5,
                                 name=f"py{tsq}_{d}")
                    for k in range(NK):
                        nc.tensor.matmul(
                            py, lhsT=wfo_sb[k][:, d * P:(d + 1) * P],
                            rhs=h_sb[k][:, qcol],
                            start=(k == 0), stop=False)
                        yield
                    nc.tensor.matmul(
                        py, lhsT=wao_sb[:, d * P:(d + 1) * P],
                        rhs=out_sb[:, qcol], start=False, stop=True)
                    y_sb = yp.tile([P, TS], f32, tag="ysb",
                                   name=f"ysb{tsq}_{d}")
                    nc.vector.tensor_copy(y_sb, py)
                    nc.gpsimd.dma_start(out=yT_d[d * P:(d + 1) * P, qcol],
                                        in_=y_sb)

                def y_step(n):
                    done = 0
                    while done < n and y_chains:
                        try:
                            next(y_chains[0])
                        except StopIteration:
                            y_chains.pop(0)
                        done += 1

                def emit_sims_pair(tsq, kt):
                    qcol = slice(tsq * TS, (tsq + 1) * TS)
                    kcols = slice(kt * P, (kt + 1) * P)
                    for h in range(LH):
                        psim = ps.tile([P, TS], f32, tag="pp", bufs=5,
                                       name=f"psim{tsq}_{h}_{kt}")
                        if h == 0:
                            nc.tensor.matmul(psim, lhsT=kv_sb[0:64, kcols],
                                             rhs=q2_sb[0:64, qcol],
                                             start=True, stop=True)
                        else:
                            nc.tensor.matmul(psim, lhsT=k2_sb[64:128, kcols],
                                             rhs=q2_sb[64:128, qcol],
                                             start=True, stop=True)
                        es = esp.tile([P, TS], bf16, tag="es",
                                      name=f"es{tsq}_{h}_{kt}")
                        nc.scalar.activation(es, psim, AF.Exp, bias=zero_col)
                        es_store[(tsq, h, kt)] = es

                def av_mm(tsq, h, kt):
                    if kt == 0:
                        pavs[(tsq, h)] = ps.tile([P, TS], f32, tag="pav",
                                                 bufs=3, name=f"pav{tsq}_{h}")
                    nc.tensor.matmul(
                        pavs[(tsq, h)][0:65, :], lhsT=v_aug[kt][:, 0:65],
                        rhs=es_store.pop((tsq, h, kt)),
                        start=(kt == 0), stop=(kt == NKT - 1))

                def emit_av_epilogue(tsq, h):
                    b = tsq * LH + h
                    qcol = slice(tsq * TS, (tsq + 1) * TS)
                    pav = pavs.pop((tsq, h))
                    # denominator (partition 64) -> reciprocal (stays at
                    # partition 64) -> K=1 fp32 PE broadcast over 64 rows,
                    # reading the stationary+moving operands at partition 64
                    rec64 = atmp.tile([P, TS], mybir.dt.float32r,
                                      tag="rec64")
                    with nc.allow_low_precision(
                            reason="f32r broadcast operand; ~19-bit "
                                   "mantissa is plenty for 1/denom"):
                        nc.vector.reciprocal(rec64[64:65, :],
                                             pav[64:65, :])
                    pB = ps.tile([64, TS], f32, tag="pp", bufs=5,
                                 name=f"pB{b}")
                    nc.tensor.matmul(pB, lhsT=ones_hi[64:65, :],
                                     rhs=rec64[64:65, :],
                                     start=True, stop=True)
                    rb = atmp.tile([64, TS], f32, tag="rb")
                    nc.vector.tensor_copy(rb, pB)
                    if h == 0:
                        nc.vector.tensor_mul(out_sb[0:64, qcol],
                                             pav[0:64, :], rb)
                    else:
                        oh1 = atmp.tile([64, TS], bf16, tag="oh1")
                        nc.vector.tensor_mul(oh1, pav[0:64, :], rb)
                        nc.gpsimd.dma_start(out=out_sb[64:128, qcol], in_=oh1)
                        if tsq == 0:
                            # slice-0 attn merges; then slice-1 ff chains
                            # (they reuse the yff tiles slice 0 just drained)
                            y_chains.extend(y_attn_chain_gen(0, d)
                                            for d in range(NK))
                            y_chains.extend(y_ff_chain_gen(1, d)
                                            for d in range(NK))
                        elif tsq == 1:
                            y_chains.extend(y_attn_chain_gen(1, d)
                                            for d in range(NK))
                        else:
                            y_chains.extend(y_chain_gen(tsq, d)
                                            for d in range(NK))

                # slot 0: sims, with slice-0 ff chains as PE filler
                y_chains.extend(y_ff_chain_gen(0, d) for d in range(NK))
                for kt in range(NKT):
                    emit_sims_pair(0, kt)
                    y_step(2)
                # slots 1..NTS-1
                for b in range(1, NTS):
                    for kt in range(NKT):
                        emit_sims_pair(b, kt)
                        av_mm(b - 1, 0, kt)
                        if kt == NKT - 1:
                            emit_av_epilogue(b - 1, 0)
                        av_mm(b - 1, 1, kt)
                        y_step(3)
                    emit_av_epilogue(b - 1, 1)
                # final AV slot
                for kt in range(NKT):
                    av_mm(NTS - 1, 0, kt)
                    av_mm(NTS - 1, 1, kt)
                    y_step(3)
                emit_av_epilogue(NTS - 1, 0)
                emit_av_epilogue(NTS - 1, 1)
                y_step(1 << 30)

    nc.compile()
    return nc


def _get_nc():
    if "nc" not in _STATE:
        _STATE["nc"] = _build_nc()
    return _STATE["nc"]


def _prep_inputs(x, gamma, w_fused, w_attn_out, w_ff_out):
    """Host-side shard packing. Returns in_maps for the 8 cores."""
    x = np.asarray(x, dtype=np.float32)
    gamma = np.asarray(gamma, dtype=np.float32)
    w_fused = np.asarray(w_fused, dtype=np.float32)
    w_attn_out = np.asarray(w_attn_out, dtype=np.float32)
    w_ff_out = np.asarray(w_ff_out, dtype=np.float32)

    # fold gamma into w_fused rows; fold q scale into q columns
    wf = w_fused * gamma[:, None]
    wf = wf.copy()
    wf[:, :ATTN_INNER] *= DH ** -0.5

    q_blk = wf[:, :ATTN_INNER]
    k_blk = wf[:, ATTN_INNER:ATTN_INNER + DH]
    v_blk = wf[:, ATTN_INNER + DH:ATTN_INNER + 2 * DH]
    ffx_blk = wf[:, ATTN_INNER + 2 * DH:ATTN_INNER + 2 * DH + FF_INNER]
    gate_blk = wf[:, ATTN_INNER + 2 * DH + FF_INNER:]

    xT = [np.ascontiguousarray(x[b].T).astype(_BF16) for b in range(B)]

    in_maps = []
    for c in range(NCORES):
        b, s = divmod(c, TP)
        cols = [q_blk[:, P * s:P * s + P], k_blk, v_blk]
        for j in range(NK):
            cols.append(gate_blk[:, FF_SH * s + j * P: FF_SH * s + (j + 1) * P])
            cols.append(ffx_blk[:, FF_SH * s + j * P: FF_SH * s + (j + 1) * P])
        wf_c = np.concatenate(cols, axis=1).astype(_BF16)
        wao_c = np.ascontiguousarray(
            w_attn_out[P * s:P * s + P, :]).astype(_BF16)
        wfo_c = np.ascontiguousarray(
            w_ff_out[FF_SH * s:FF_SH * (s + 1), :]).astype(_BF16)
        in_maps.append({"xT": xT[b], "wf": wf_c, "wao": wao_c, "wfo": wfo_c})
    return in_maps


def kernel(x, gamma, w_fused, w_attn_out, w_ff_out):
    import time
    from concourse.bass_utils import run_bass_kernel_spmd

    nc = _get_nc()
    in_maps = _prep_inputs(x, gamma, w_fused, w_attn_out, w_ff_out)

    t0 = time.perf_counter()
    res = run_bass_kernel_spmd(nc, in_maps, core_ids=list(range(NCORES)))
    t1 = time.perf_counter()
    _STATE["last_wall_ns"] = (t1 - t0) * 1e9

    y = np.empty((B, N, D), dtype=np.float32)
    for b in range(B):
        acc = res.results[b * TP]["yT"].astype(np.float32)
        for s in range(1, TP):
            acc = acc + res.results[b * TP + s]["yT"]
        y[b] = acc.T
    return y


# revision 55
# speedup vs baseline: 1.0915x; 1.0011x over previous
"""Fused transformer block (LN + fused QKV/FF proj + MQA attention + SwiGLU FF)
on 8 TRN2 NeuronCores.

Sharding: hybrid DP2 x TP4.
  core c -> batch b = c//4, tensor-parallel shard s = c%4.
  Each core handles its batch's full 2048 tokens in feature-major layout:
    - q: 2 of 8 heads (cols 128*s .. 128*s+128 of the q block)
    - k/v: replicated (width 64 each)
    - ff: 1024 of 4096 cols of both ff_x and gate
    - attn_out / ff_out: matching row shards -> partial [1024, 2048] outputs
  Host sums the 4 partial outputs per batch (row-parallel reduction).

Device layout is feature-major (features on SBUF partitions, tokens on the
free dim) so every matmul contraction is over the partition dim.  gamma and
the q-scale (dim_head**-0.5) are folded into w_fused on the host; per-token
LayerNorm mu/rstd are computed on device via ones-vector matmuls, broadcast
across partitions through a DRAM bounce, mean-subtract applied in place on
x^T, and rstd folded into each projection's PSUM evacuation.
"""

import numpy as np
import ml_dtypes

# ---- problem shapes (hardcoded) ----
B, N, D = 2, 2048, 1024
DH = 64
HEADS = 8
ATTN_INNER = HEADS * DH          # 512
FF_INNER = 4 * D                 # 4096
T = N                            # tokens per core
P = 128
TS = 512
NTS = T // TS                    # 4
NK = D // P                      # 8
NCORES = 8
TP = 4
LH = HEADS // TP                 # 2 local heads
FF_SH = FF_INNER // TP           # 1024
FSH = LH * DH + 2 * DH + 2 * FF_SH   # 2304 packed proj cols per core
NF = FSH // P                    # 18
NKT = T // P                     # 16 key-token tiles

_BF16 = ml_dtypes.bfloat16

_STATE = {}


def _build_nc():
    from concourse import bacc
    import concourse.tile as tile
    from concourse.tile import add_dep_helper
    import concourse.mybir as mybir

    bf16 = mybir.dt.bfloat16
    f32 = mybir.dt.float32
    AF = mybir.ActivationFunctionType

    nc = bacc.Bacc("TRN2", target_bir_lowering=False, debug=False)

    xT_d = nc.dram_tensor("xT", [D, T], bf16, kind="ExternalInput")
    wf_d = nc.dram_tensor("wf", [D, FSH], bf16, kind="ExternalInput")
    wao_d = nc.dram_tensor("wao", [P, D], bf16, kind="ExternalInput")
    wfo_d = nc.dram_tensor("wfo", [D, D], bf16, kind="ExternalInput")
    yT_d = nc.dram_tensor("yT", [D, T], f32, kind="ExternalOutput")
    # DRAM bounce rows for partition-broadcast
    mu_d = nc.dram_tensor("mu_bounce", [1, T], bf16)
    rstd_d = nc.dram_tensor("rstd_bounce", [1, T], f32)

    with tile.TileContext(nc) as tc:
        with (
            tc.tile_pool(name="cp", bufs=1) as cp,
            tc.tile_pool(name="wp", bufs=1) as wp,
            tc.tile_pool(name="acts", bufs=1) as acts,
            tc.tile_pool(name="rows", bufs=1) as rows,
            tc.tile_pool(name="tmp", bufs=4) as tmp,
            tc.tile_pool(name="ps", bufs=1, space="PSUM") as ps,
        ):
            # ---- constants ----
            ones_col = cp.tile([P, 1], bf16)
            nc.vector.memset(ones_col, 1.0)
            ones_row_bf = cp.tile([1, P], bf16)
            nc.vector.memset(ones_row_bf, 1.0)
            f32r = mybir.dt.float32r
            ones_hi = cp.tile([P, 64], f32r)
            nc.vector.memset(ones_hi.bitcast(f32), 1.0)
            ones_row_r = cp.tile([1, P], f32r)
            nc.vector.memset(ones_row_r.bitcast(f32), 1.0)
            eps_t = cp.tile([1, 1], f32)
            nc.vector.memset(eps_t, 1e-5)
            zero_col = cp.tile([P, 1], f32)
            nc.vector.memset(zero_col, 0.0)
            # warm the ACT function tables before the bulk DMAs queue up:
            # lazy table loads otherwise serialize behind ~9MB of weight
            # traffic and stall the first LayerNorm square by ~8us
            warm_o = cp.tile([1, 4], f32)
            for wi, fn in enumerate((AF.Sigmoid, AF.Exp, AF.Sqrt)):
                nc.scalar.activation(warm_o[0:1, wi:wi + 1],
                                     eps_t, fn, bias=zero_col[0:1, :])

            # ---- persistent activations ----
            q2_sb = acts.tile([P, T], bf16)    # q both local heads, feature-major
            kv_sb = acts.tile([P, T], bf16)    # k rows 0-63, v rows 64-127
            k2_sb = acts.tile([P, T], bf16)    # k replica at partitions 64-127
                                               # (head-1 sim row-group packing)
            h_sb = [acts.tile([P, T], bf16, name=f"h{j}") for j in range(NK)]
            out_sb = acts.tile([P, T], bf16)   # attention out, both heads
            mu_b = acts.tile([P, T], bf16)     # mu broadcast
            rstd_b = acts.tile([P, T], f32)    # rstd broadcast
            v_aug = [acts.tile([P, 72], bf16, name=f"va{kt}") for kt in range(NKT)]
            for kt in range(NKT):
                nc.vector.memset(v_aug[kt][:, 64:65], 1.0)

            with tc.tile_pool(name="xp", bufs=1) as xp:
                # ---- load x^T, ts-chunked so stats/proj of slice 0 start
                # as early as possible ----
                # x in two column-halves and wf in column-quarters,
                # interleaved so that (a) slice-0/1 stats are gated on only
                # half of x and (b) the first proj chains are gated on only
                # the first quarter of wf -> the projection starts ~10us
                # earlier than with monolithic loads
                xt = [xp.tile([P, T], bf16, name=f"xt{k}") for k in range(NK)]
                wf_sb = [xp.tile([P, FSH], bf16, name=f"wf{k}")
                         for k in range(NK)]
                WQ = FSH // 4
                xh1 = slice(0, T // 2)
                xh2 = slice(T // 2, T)
                for k in range(NK):
                    nc.sync.dma_start(out=xt[k][:, xh1],
                                      in_=xT_d[k * P:(k + 1) * P, xh1])
                for q in range(2):
                    qc = slice(q * WQ, (q + 1) * WQ)
                    for k in range(NK):
                        nc.sync.dma_start(out=wf_sb[k][:, qc],
                                          in_=wf_d[k * P:(k + 1) * P, qc])
                for k in range(NK):
                    nc.sync.dma_start(out=xt[k][:, xh2],
                                      in_=xT_d[k * P:(k + 1) * P, xh2])
                for q in range(2, 4):
                    qc = slice(q * WQ, (q + 1) * WQ)
                    for k in range(NK):
                        nc.sync.dma_start(out=wf_sb[k][:, qc],
                                          in_=wf_d[k * P:(k + 1) * P, qc])
                # ---- LayerNorm statistics, per token-slice ----
                def emit_stats(ts):
                    col = slice(ts * TS, (ts + 1) * TS)
                    ps_s = ps.tile([1, TS], f32, tag="pp", bufs=5,
                                   name=f"ps_s{ts}")
                    for k in range(NK):
                        nc.tensor.matmul(ps_s, lhsT=ones_col,
                                         rhs=xt[k][:, col],
                                         start=(k == 0), stop=(k == NK - 1))
                    ps_s2 = ps.tile([1, TS], f32, tag="pp", bufs=5,
                                    name=f"ps_s2{ts}")
                    for k in range(NK):
                        x2t = tmp.tile([P, TS], bf16, tag="x2t")
                        nc.vector.tensor_mul(x2t, xt[k][:, col], xt[k][:, col])
                        nc.tensor.matmul(ps_s2, lhsT=ones_col, rhs=x2t,
                                         start=(k == 0), stop=(k == NK - 1))
                    # negvarD = (s^2)/D - s2 = -D*var ;  std = sqrt(-negvarD/D
                    # + eps) ;  mu(bf16) = s/D
                    # square on DVE (copy to SBUF + mul): keeps the ACT
                    # Sqrt function table resident instead of thrashing
                    # Square/Sqrt table sets every slice (1.28us per reload)
                    ssq_sb = rows.tile([1, TS], f32, tag="ssq_sb")
                    nc.vector.tensor_copy(ssq_sb, ps_s)
                    ssq = rows.tile([1, TS], f32, tag="ssq")
                    nc.vector.tensor_mul(ssq, ssq_sb, ps_s)
                    mu_bf_r = rows.tile([1, TS], bf16, tag="mu_bf_r")
                    nc.vector.tensor_scalar_mul(mu_bf_r, ps_s, 1.0 / D)
                    negvar = rows.tile([1, TS], f32, tag="negvar")
                    nc.vector.scalar_tensor_tensor(
                        negvar, ssq, 1.0 / D, ps_s2,
                        op0=mybir.AluOpType.mult,
                        op1=mybir.AluOpType.subtract)
                    std = rows.tile([1, TS], f32, tag="std")
                    nc.scalar.activation(std, negvar, AF.Sqrt, bias=eps_t,
                                         scale=-1.0 / D)
                    rstd_r = rows.tile([1, TS],
                                       f32r if ts == 0 else f32,
                                       tag="rstd_r")
                    with nc.allow_low_precision(
                            reason="f32r broadcast operand; ~19-bit "
                                   "mantissa is plenty for rstd"):
                        nc.vector.reciprocal(rstd_r, std)
                    if ts == 0:
                        # slice 0 gates the whole projection phase: broadcast
                        # via K=1 PE matmuls (DMA-free; the DRAM-bounce path
                        # would queue behind the bulk weight loads)
                        pmu = ps.tile([P, TS], f32, tag="pp", bufs=5,
                                      name="pmu0")
                        nc.tensor.matmul(pmu, lhsT=ones_row_bf[0:1, :],
                                         rhs=mu_bf_r, start=True, stop=True)
                        nc.vector.tensor_copy(mu_b[:, col], pmu)
                        prs = ps.tile([P, TS], f32, tag="pp", bufs=5,
                                      name="prs0")
                        nc.tensor.matmul(prs, lhsT=ones_row_r[0:1, :],
                                         rhs=rstd_r,
                                         start=True, stop=True)
                        nc.vector.tensor_copy(rstd_b[:, col], prs)
                    else:
                        # bounce rows through DRAM, broadcast to 128
                        # partitions (gpsimd SWDGE queue; lands during the
                        # previous slice's projection)
                        nc.gpsimd.dma_start(out=mu_d[0:1, col], in_=mu_bf_r)
                        nc.gpsimd.dma_start(out=rstd_d[0:1, col], in_=rstd_r)
                        nc.gpsimd.dma_start(
                            out=mu_b[:, col],
                            in_=mu_d[0:1, col].partition_broadcast(P))
                        nc.gpsimd.dma_start(
                            out=rstd_b[:, col],
                            in_=rstd_d[0:1, col].partition_broadcast(P))

                gate = {}

                def emit_center(ts):
                    # center x in place (emitted separately: this blocks DVE
                    # on the mu broadcast, so it must come after all stats
                    # squares that later PE chains depend on)
                    col = slice(ts * TS, (ts + 1) * TS)
                    for k in range(NK):
                        nc.vector.tensor_sub(xt[k][:, col], xt[k][:, col],
                                             mu_b[:, col])

                # ---- fused projection ----
                # packed col order: [q(128) | kv(128) | (gate_j, ffx_j) x 8]
                def emit_proj(ts):
                    col = slice(ts * TS, (ts + 1) * TS)
                    cur_silu = None
                    for fi in range(NF):
                        pp = ps.tile([P, TS], f32, tag="pp", bufs=5,
                                     name=f"pp{ts}_{fi}")
                        for k in range(NK):
                            nc.tensor.matmul(
                                pp,
                                lhsT=wf_sb[k][:, fi * P:(fi + 1) * P],
                                rhs=xt[k][:, col],
                                start=(k == 0), stop=(k == NK - 1))
                        if fi == 0:
                            nc.vector.tensor_mul(q2_sb[:, col], pp,
                                                 rstd_b[:, col])
                        elif fi == 1:
                            nc.vector.tensor_mul(kv_sb[:, col], pp,
                                                 rstd_b[:, col])
                            # replicate k rows to partitions 64-127 so head-1
                            # sims run in the upper PE row group
                            nc.sync.dma_start(out=k2_sb[64:128, col],
                                               in_=kv_sb[0:64, col])
                        elif fi % 2 == 0:  # gate_j
                            g = tmp.tile([P, TS], bf16, tag="g")
                            nc.vector.tensor_mul(g, pp, rstd_b[:, col])
                            sg = tmp.tile([P, TS], bf16, tag="sg")
                            nc.scalar.activation(sg, g, AF.Sigmoid, bias=zero_col)
                            silu = tmp.tile([P, TS], bf16, tag="silu")
                            nc.vector.tensor_mul(silu, g, sg)
                            cur_silu = silu
                        else:  # ffx_j
                            j = (fi - 3) // 2
                            fx = tmp.tile([P, TS], bf16, tag="fx")
                            nc.vector.tensor_mul(fx, pp, rstd_b[:, col])
                            hmul = nc.vector.tensor_mul(h_sb[j][:, col],
                                                        cur_silu, fx)
                            if ts == 1 and fi == NF - 1:
                                gate["i"] = hmul
                    # v -> token-major, into the v_aug tiles for this slice
                    for kt in range(ts * (TS // P), (ts + 1) * (TS // P)):
                        nc.sync.dma_start(
                            out=v_aug[kt][:, 0:64],
                            in_=kv_sb[64:128, kt * P:(kt + 1) * P],
                            transpose=True)

                # schedule: only stats(0) ahead of proj(0); later slices'
                # stats (and their row math / broadcasts / centering) hide
                # under the previous slice's projection
                emit_stats(0)
                emit_stats(1)
                emit_center(0)
                emit_proj(0)
                emit_stats(2)
                emit_center(1)
                emit_proj(1)
                emit_stats(3)
                emit_center(2)
                emit_proj(2)
                emit_center(3)
                emit_proj(3)
                # output-side weights: needed only ~190us in.  Explicitly
                # gated behind the end of proj slice 1 so the scheduler does
                # not hoist these (dependency-free) DMAs ahead of the x/wf
                # loads and halve the effective prologue load bandwidth.
                wao_sb = wp.tile([P, D], bf16)
                w_in = nc.gpsimd.dma_start(out=wao_sb, in_=wao_d[:, :])
                add_dep_helper(w_in.ins, gate["i"].ins,
                               reason="defer wao load")
                wfo_sb = []
                for k in range(NK):
                    t_ = wp.tile([P, D], bf16, name=f"wfo{k}")
                    w_in = nc.gpsimd.dma_start(out=t_,
                                               in_=wfo_d[k * P:(k + 1) * P, :])
                    add_dep_helper(w_in.ins, gate["i"].ins,
                                   reason="defer wfo load")
                    wfo_sb.append(t_)

            # xp closed: x/wf tiles are dead, reuse SBUF for attention tiles.
            # Attention pipeline over tsq-slots.  Head-0 sims run in PE rows
            # 0-63, head-1 sims concurrently in rows 64-127 (k replicated at
            # partitions 64-127, q head 1 already there).  AV matmuls of the
            # previous slot and y-chain matmuls interleave at ~exp rate so
            # the PE stays busy while ACT churns the exps.
            with (
                tc.tile_pool(name="esp", bufs=44) as esp,
                tc.tile_pool(name="atmp", bufs=3) as atmp,
                tc.tile_pool(name="yp", bufs=4) as yp,
                tc.tile_pool(name="yffp", bufs=1) as yffp,
            ):
                es_store = {}
                pavs = {}
                y_chains = []

                # Slice 0's output chains are split: the ff-only part runs
                # during attention slots 0-1 (when no other y work is
                # unlocked yet and the PE would otherwise wait on ACT exps),
                # accumulating to SBUF; the single attn matmul merges in
                # during evacuation once slice 0's attention output exists.
                yff_sb = [yffp.tile([P, TS], f32, name=f"yff{d}")
                          for d in range(NK)]

                def y_ff_chain_gen(tsq, d):
                    qcol = slice(tsq * TS, (tsq + 1) * TS)
                    py = ps.tile([P, TS], f32, tag="pp", bufs=5,
                                 name=f"pyf{tsq}_{d}")
                    for k in range(NK):
                        nc.tensor.matmul(
                            py, lhsT=wfo_sb[k][:, d * P:(d + 1) * P],
                            rhs=h_sb[k][:, qcol],
                            start=(k == 0), stop=(k == NK - 1))
                        yield
                    nc.vector.tensor_copy(yff_sb[d], py)

                def y_attn_chain_gen(tsq, d):
                    qcol = slice(tsq * TS, (tsq + 1) * TS)
                    pa = ps.tile([P, TS], f32, tag="pp", bufs=5,
                                 name=f"pya{tsq}_{d}")
                    nc.tensor.matmul(pa, lhsT=wao_sb[:, d * P:(d + 1) * P],
                                     rhs=out_sb[:, qcol],
                                     start=True, stop=True)
                    yield
                    y_sb = yp.tile([P, TS], f32, tag="ysb",
                                   name=f"ysba{tsq}_{d}")
                    nc.vector.scalar_tensor_tensor(
                        y_sb, pa, 1.0, yff_sb[d],
                        op0=mybir.AluOpType.mult,
                        op1=mybir.AluOpType.add)
                    nc.gpsimd.dma_start(out=yT_d[d * P:(d + 1) * P, qcol],
                                        in_=y_sb)

                def y_chain_gen(tsq, d):
                    qcol = slice(tsq * TS, (tsq + 1) * TS)
                    py = ps.tile([P, TS], f32, tag="pp", bufs=5,
                                 name=f"py{tsq}_{d}")
                    for k in range(NK):
                        nc.tensor.matmul(
                            py, lhsT=wfo_sb[k][:, d * P:(d + 1) * P],
                            rhs=h_sb[k][:, qcol],
                            start=(k == 0), stop=False)
                        yield
                    nc.tensor.matmul(
                        py, lhsT=wao_sb[:, d * P:(d + 1) * P],
                        rhs=out_sb[:, qcol], start=False, stop=True)
                    y_sb = yp.tile([P, TS], f32, tag="ysb",
                                   name=f"ysb{tsq}_{d}")
                    nc.vector.tensor_copy(y_sb, py)
                    nc.gpsimd.dma_start(out=yT_d[d * P:(d + 1) * P, qcol],
                                        in_=y_sb)

                def y_step(n):
                    done = 0
                    while done < n and y_chains:
                        try:
                            next(y_chains[0])
                        except StopIteration:
                            y_chains.pop(0)
                        done += 1

                def emit_sims_pair(tsq, kt):
                    qcol = slice(tsq * TS, (tsq + 1) * TS)
                    kcols = slice(kt * P, (kt + 1) * P)
                    for h in range(LH):
                        psim = ps.tile([P, TS], f32, tag="pp", bufs=5,
                                       name=f"psim{tsq}_{h}_{kt}")
                        if h == 0:
                            nc.tensor.matmul(psim, lhsT=kv_sb[0:64, kcols],
                                             rhs=q2_sb[0:64, qcol],
                                             start=True, stop=True)
                        else:
                            nc.tensor.matmul(psim, lhsT=k2_sb[64:128, kcols],
                                             rhs=q2_sb[64:128, qcol],
                                             start=True, stop=True)
                        es = esp.tile([P, TS], bf16, tag="es",
                                      name=f"es{tsq}_{h}_{kt}")
                        nc.scalar.activation(es, psim, AF.Exp, bias=zero_col)
                        es_store[(tsq, h, kt)] = es

                def av_mm(tsq, h, kt):
                    if kt == 0:
                        pavs[(tsq, h)] = ps.tile([P, TS], f32, tag="pav",
                                                 bufs=3, name=f"pav{tsq}_{h}")
                    nc.tensor.matmul(
                        pavs[(tsq, h)][0:65, :], lhsT=v_aug[kt][:, 0:65],
                        rhs=es_store.pop((tsq, h, kt)),
                        start=(kt == 0), stop=(kt == NKT - 1))

                def emit_av_epilogue(tsq, h):
                    b = tsq * LH + h
                    qcol = slice(tsq * TS, (tsq + 1) * TS)
                    pav = pavs.pop((tsq, h))
                    # denominator (partition 64) -> reciprocal (stays at
                    # partition 64) -> K=1 fp32 PE broadcast over 64 rows,
                    # reading the stationary+moving operands at partition 64
                    rec64 = atmp.tile([P, TS], mybir.dt.float32r,
                                      tag="rec64")
                    with nc.allow_low_precision(
                            reason="f32r broadcast operand; ~19-bit "
                                   "mantissa is plenty for 1/denom"):
                        nc.vector.reciprocal(rec64[64:65, :],
                                             pav[64:65, :])
                    pB = ps.tile([64, TS], f32, tag="pp", bufs=5,
                                 name=f"pB{b}")
                    nc.tensor.matmul(pB, lhsT=ones_hi[64:65, :],
                                     rhs=rec64[64:65, :],
                                     start=True, stop=True)
                    rb = atmp.tile([64, TS], f32, tag="rb")
                    nc.vector.tensor_copy(rb, pB)
                    if h == 0:
                        nc.vector.tensor_mul(out_sb[0:64, qcol],
                                             pav[0:64, :], rb)
                    else:
                        oh1 = atmp.tile([64, TS], bf16, tag="oh1")
                        nc.vector.tensor_mul(oh1, pav[0:64, :], rb)
                        nc.gpsimd.dma_start(out=out_sb[64:128, qcol], in_=oh1)
                        if tsq == 0:
                            # slice-0 attn merges; then slice-1 ff chains
                            # (they reuse the yff tiles slice 0 just drained)
                            y_chains.extend(y_attn_chain_gen(0, d)
                                            for d in range(NK))
                            y_chains.extend(y_ff_chain_gen(1, d)
                                            for d in range(NK))
                        elif tsq == 1:
                            y_chains.extend(y_attn_chain_gen(1, d)
                                            for d in range(NK))
                        else:
                            y_chains.extend(y_chain_gen(tsq, d)
                                            for d in range(NK))

                # slot 0: sims, with slice-0 ff chains as PE filler
                y_chains.extend(y_ff_chain_gen(0, d) for d in range(NK))
                for kt in range(NKT):
                    emit_sims_pair(0, kt)
                    y_step(2)
                # slots 1..NTS-1
                for b in range(1, NTS):
                    for kt in range(NKT):
                        emit_sims_pair(b, kt)
                        av_mm(b - 1, 0, kt)
                        if kt == NKT - 1:
                            emit_av_epilogue(b - 1, 0)
                        av_mm(b - 1, 1, kt)
                        y_step(3)
                    emit_av_epilogue(b - 1, 1)
                # final AV slot
                for kt in range(NKT):
                    av_mm(NTS - 1, 0, kt)
                    av_mm(NTS - 1, 1, kt)
                    y_step(3)
                emit_av_epilogue(NTS - 1, 0)
                emit_av_epilogue(NTS - 1, 1)
                y_step(1 << 30)

    nc.compile()
    return nc


def _get_nc():
    if "nc" not in _STATE:
        _STATE["nc"] = _build_nc()
    return _STATE["nc"]


def _prep_inputs(x, gamma, w_fused, w_attn_out, w_ff_out):
    """Host-side shard packing. Returns in_maps for the 8 cores."""
    x = np.asarray(x, dtype=np.float32)
    gamma = np.asarray(gamma, dtype=np.float32)
    w_fused = np.asarray(w_fused, dtype=np.float32)
    w_attn_out = np.asarray(w_attn_out, dtype=np.float32)
    w_ff_out = np.asarray(w_ff_out, dtype=np.float32)

    # fold gamma into w_fused rows; fold q scale into q columns
    wf = w_fused * gamma[:, None]
    wf = wf.copy()
    wf[:, :ATTN_INNER] *= DH ** -0.5

    q_blk = wf[:, :ATTN_INNER]
    k_blk = wf[:, ATTN_INNER:ATTN_INNER + DH]
    v_blk = wf[:, ATTN_INNER + DH:ATTN_INNER + 2 * DH]
    ffx_blk = wf[:, ATTN_INNER + 2 * DH:ATTN_INNER + 2 * DH + FF_INNER]
    gate_blk = wf[:, ATTN_INNER + 2 * DH + FF_INNER:]

    xT = [np.ascontiguousarray(x[b].T).astype(_BF16) for b in range(B)]

    in_maps = []
    for c in range(NCORES):
        b, s = divmod(c, TP)
        cols = [q_blk[:, P * s:P * s + P], k_blk, v_blk]
        for j in range(NK):
            cols.append(gate_blk[:, FF_SH * s + j * P: FF_SH * s + (j + 1) * P])
            cols.append(ffx_blk[:, FF_SH * s + j * P: FF_SH * s + (j + 1) * P])
        wf_c = np.concatenate(cols, axis=1).astype(_BF16)
        wao_c = np.ascontiguousarray(
            w_attn_out[P * s:P * s + P, :]).astype(_BF16)
        wfo_c = np.ascontiguousarray(
            w_ff_out[FF_SH * s:FF_SH * (s + 1), :]).astype(_BF16)
        in_maps.append({"xT": xT[b], "wf": wf_c, "wao": wao_c, "wfo": wfo_c})
    return in_maps


def kernel(x, gamma, w_fused, w_attn_out, w_ff_out):
    import time
    from concourse.bass_utils import run_bass_kernel_spmd

    nc = _get_nc()
    in_maps = _prep_inputs(x, gamma, w_fused, w_attn_out, w_ff_out)

    t0 = time.perf_counter()
    res = run_bass_kernel_spmd(nc, in_maps, core_ids=list(range(NCORES)))
    t1 = time.perf_counter()
    _STATE["last_wall_ns"] = (t1 - t0) * 1e9

    y = np.empty((B, N, D), dtype=np.float32)
    for b in range(B):
        acc = res.results[b * TP]["yT"].astype(np.float32)
        for s in range(1, TP):
            acc = acc + res.results[b * TP + s]["yT"]
        y[b] = acc.T
    return y
